# revision 21
# baseline (speedup 1.0000x reference)
"""Multi-head attention (B=4, S=2048, D=512, H=8) on 8 trn2 cores.

Sharding: core c handles batch b=c//2 and the head-quad qh=c%2 (heads
4*qh..4*qh+3). Each core computes q/k/v projections for its 4 heads over the
full sequence, flash-style attention (scores kept transposed [j, i] so all
matmul contractions land on the partition dim with zero on-device transposes),
and the partial output projection over its 256 o-dims. The host pre-packs
x/weight slices into sbuf-layout 2D dram tensors (free) and sums/transposes
the two partial outputs per batch.

Design (single fused pipeline; HW trace shows ACT is the steady-state
metronome at ~1.08us/slot with PE just underneath it):
 - The scalar engine's exp is the hard floor: 128 exp tiles of [128,1024]
   ~= 139us/core busy. The schedule keeps ACT saturated: everything else
   (projections, output projection, normalization) hides in PE/DVE slack.
 - Attention inner loop is software-pipelined with the PE stream ordered
   [scores(jc), AV(jc-3), deferred-quantum]; at bufs=10 decouples the exp
   WAR from AV jitter at unit boundaries.
 - Input DMAs: host packs weights and x into contiguous [128, N] dram
   tensors matching sbuf layout exactly, so the whole input stream is 7
   triggers with 4-8KB descriptors, ordered by first use (wkq -> x quarters
   -> wv -> wo). First exp fires ~9us in instead of ~24us.
 - The q/k/v projections and the output projection are cut into quanta and
   drip-fed into the attention loop's PE slack from a deadline-sorted queue
   with staggered deadlines (~1 quantum/slot, no bursts). PSUM: sp
   [128,1024]x2 + op [128,1024]x1 + scratch [128,512]x2 = exactly 8 banks.
 - Softmax normalization without DRAM round-trips: each v block carries 64
   ones-columns ([128,128] stationary = 64 ones | 64 v), so the AV matmul
   replicates the softmax denominator into op psum rows 0..63 for free. The
   epilogue reads op PSUM directly: reciprocal_approx_fast (base-0
   partitions, 18-bit exact; sums are ~[1,1e20], far from its denorm/inf
   edge cases) then one multiply. No drain copies - the next unit's AV
   start=True write WAR-waits on these two reads, same latency as the old
   copy pair but 2 fewer DVE ops per unit.
 - Unit boundaries: previous unit's last AVs drain 2/slot at jc 0-1, its
   epilogue is emitted at jc==2 (one slot before the op psum is re-acquired
   at jc==3), shrinking the boundary stall.
 - fp16 for the score path (x, w_qkv, q, k, w_out, o): 1 cycle/row on the PE
   like bf16 but 8x the mantissa (bf16 q/k fails the 2e-2 gate at ~2.2e-2;
   fp16 lands at 3.3e-3). exp output (attn weights) stays bf16 for fp32
   exponent range since softmax skips max-subtraction, psum stays fp32.
   Output partials are written fp16 (host sums in fp32): halves the output
   DMA and the tail transfer.
"""
import sys

sys.path.insert(0, "/opt/trn_rl_repo")
import numpy as np

B, S, D, H, HD = 4, 2048, 512, 8, 64
HPC = 4          # heads per core
DQ = HPC * HD    # 256 projection dims per core
NCORES = 8
VW = 2 * HD      # v block width: 64 v-dims + 64 ones columns (128)
IH = S // 2      # i-half processed per attention unit (1024)
AV_LAG = 4       # attn@v trails scores by this many j-chunks

_cache = {}


def _build_nc():
    import concourse.bacc as bacc
    import concourse.mybir as mybir
    import concourse.tile as tile

    F32 = mybir.dt.float32
    F16, BF16 = mybir.dt.float16, mybir.dt.bfloat16
    EXP = mybir.ActivationFunctionType.Exp

    nc = bacc.Bacc("TRN2", target_bir_lowering=False, debug=False)

    # host-packed inputs, all contiguous 2D [128, N] in final sbuf layout:
    # wkqA/B: [k d-blocks (4x128) | q d-blocks (4x128)] for head-pair m=0/1
    # wvT: [v d-blocks (4x256)]
    # woT: [wo kc-blocks (2x512)]
    # xQ0..3: x column-quarters, d-major blocks of 512 cols each
    wkqAT = nc.dram_tensor("wkqAT", [128, 1024], F16, kind="ExternalInput")
    wkqBT = nc.dram_tensor("wkqBT", [128, 1024], F16, kind="ExternalInput")
    wvT = nc.dram_tensor("wvT", [128, 1024], F16, kind="ExternalInput")
    woT = nc.dram_tensor("woT", [128, 1024], F16, kind="ExternalInput")
    xQ = [nc.dram_tensor(f"xQ{r}", [128, 2048], F16, kind="ExternalInput")
          for r in range(4)]
    # output: block m (cols m*2048+s) holds out-dims m*128..m*128+127
    outT = nc.dram_tensor("outT", [128, 8192], F16, kind="ExternalOutput")

    with tile.TileContext(nc) as tc:
        with tc.tile_pool(name="sb", bufs=1) as sb, \
             tc.tile_pool(name="ps", bufs=1, space="PSUM") as pp:
            # ---- persistent sbuf tensors ----
            wkq = [sb.tile([128, 1024], F16, tag=f"wkq{m}", name=f"wkq{m}")
                   for m in range(2)]
            wvs = sb.tile([128, 1024], F16, tag="wvs", name="wvs")
            wos = sb.tile([128, 1024], F16, tag="wos", name="wos")
            xtf = sb.tile([128, 8192], F16, tag="xtf", name="xtf")

            def wk_v(d, m):      # k weights, d-block, head-pair m
                return wkq[m][:, d * 128:(d + 1) * 128]

            def wq_v(d, m):
                return wkq[m][:, 512 + d * 128:512 + (d + 1) * 128]

            def wv_v(d, hp):     # v weights, d-block, head-pair hp
                return wvs[:, d * 256 + hp * 128:d * 256 + (hp + 1) * 128]

            def wo_v(kc, m):
                return wos[:, kc * 512 + m * 128:kc * 512 + (m + 1) * 128]

            def xts(d, c0, c1):  # x cols c0:c1 of d-block (one 512-region)
                r = c0 // 512
                assert (c1 - 1) // 512 == r
                base = r * 2048 + d * 512 + (c0 - r * 512)
                return xtf[:, base:base + (c1 - c0)]

            # ---- input DMAs ----
            # The 16 DMA engines service all queued transfers round-robin by
            # descriptor, so every queued dma_start completes only near the
            # end of the whole in-flight stream. Three levels: free-run only
            # what the first projections need (wkqA+xQ0, 0.75MB); gate wv+xQ1
            # on xQ0 and the rest on xQ1 (tiny GPSIMD copies into each gated
            # dst chain the gated DMA's WAR dependency to the completion of
            # the previous level).
            nc.sync.dma_start(out=wkq[0][:], in_=wkqAT[:, :])
            nc.sync.dma_start(out=xtf[:, 0:2048], in_=xQ[0][:, :])
            tok0 = xtf[0:1, 0:1]          # written by xQ0
            for dst in (wvs[0:1, 0:1], xtf[0:1, 2048:2049]):
                nc.gpsimd.tensor_copy(out=dst, in_=tok0)
            nc.sync.dma_start(out=wvs[:], in_=wvT[:, :])
            nc.sync.dma_start(out=xtf[:, 2048:4096], in_=xQ[1][:, :])
            tok1 = xtf[0:1, 2049:2050]    # written by xQ1 (not by tok0)
            for dst in (xtf[0:1, 4096:4097], xtf[0:1, 6144:6145],
                        wkq[1][0:1, 0:1], wos[0:1, 0:1]):
                nc.gpsimd.tensor_copy(out=dst, in_=tok1)
            nc.sync.dma_start(out=xtf[:, 4096:6144], in_=xQ[2][:, :])
            nc.sync.dma_start(out=xtf[:, 6144:8192], in_=xQ[3][:, :])
            nc.sync.dma_start(out=wkq[1][:], in_=wkqBT[:, :])
            nc.sync.dma_start(out=wos[:], in_=woT[:, :])

            qT = [sb.tile([128, S], F16, tag=f"qT{m}", name=f"qT{m}")
                  for m in range(2)]
            kT = [sb.tile([128, S], F16, tag=f"kT{m}", name=f"kT{m}")
                  for m in range(2)]
            # vv block for (jc, h): cols [0:64] = ones, [64:128] = v dims
            # (ones first so the softmax sums land at psum partitions 0:64,
            # where the base-0 custom-DVE fast reciprocal can read them)
            vv = sb.tile([128, 16 * HPC * VW], BF16, tag="vv", name="vv")
            # oTn[p]: heads (2p, 2p+1) stacked on partitions; outproj moving
            oTn = [sb.tile([128, S], F16, tag=f"oTn{p}", name=f"oTn{p}")
                   for p in range(2)]

            # ---- prologue scratch: ACT table preload + PE warm-up ----
            wub = sb.tile([128, 512], BF16, tag="wub", name="wub")
            nc.vector.memset(wub[:], 0.25)
            # tiny exp: forces the ACT exp table load off the critical path
            dummy_at = sb.tile([128, 16], BF16, tag="dummy_at", name="dummy_at")
            nc.scalar.activation(dummy_at[:], wub[:, 0:16], EXP)
            ones32 = sb.tile([128, 1], F32, tag="ones32", name="ones32")
            nc.vector.memset(ones32[:], 1.0)
            vv_ones = vv[:, :].rearrange("p (g w) -> p g w", w=VW)[:, :, 0:HD]
            nc.vector.tensor_copy(
                out=vv_ones, in_=ones32[:].to_broadcast((128, 16 * HPC, HD)))
            # keep the PE busy/ramping (HAM warm) while the critical input
            # DMAs stream in; 256-wide quanta so the tail of the warm-up
            # doesn't delay the first projection once data lands
            for _ in range(16):
                wups = pp.tile([128, 512], F32, tag="sc", bufs=2, name="wups")
                nc.tensor.matmul(wups[:, 0:256], wub[:, 0:128],
                                 wub[:, 0:256],
                                 start=True, stop=True, skip_group_check=True)

            # ---- work-group emitters ----
            def qk_cast(nm, m, sc, ps):
                tgt = (qT if nm == "q" else kT)[m]
                nc.vector.tensor_copy(
                    out=tgt[:, sc * 512:(sc + 1) * 512], in_=ps[:, 0:512])
                qk_done.add((nm, m, sc))

            def qk_group(nm, m, sc):
                """whole q/k projection group: 4 matmuls + cast (prologue)."""
                ps = pp.tile([128, 512], F32, tag="sc", bufs=2, name="ps")
                wf = wq_v if nm == "q" else wk_v
                for d in range(4):
                    nc.tensor.matmul(
                        ps[:, 0:512], wf(d, m),
                        xts(d, sc * 512, (sc + 1) * 512),
                        start=(d == 0), stop=(d == 3))
                qk_cast(nm, m, sc, ps)

            def qk_quanta(nm, m, sc, deadline):
                """same group cut into 4 one-matmul quanta for the defq,
                with staggered deadlines so pops stay ~1/slot."""
                state = {}
                def q(d):
                    def emit():
                        if d == 0:
                            state["ps"] = pp.tile([128, 512], F32, tag="sc",
                                                  bufs=2, name="psq")
                        ps = state["ps"]
                        wf = wq_v if nm == "q" else wk_v
                        nc.tensor.matmul(
                            ps[:, 0:512], wf(d, m),
                            xts(d, sc * 512, (sc + 1) * 512),
                            start=(d == 0), stop=(d == 3),
                            skip_group_check=True)
                        if d == 3:
                            qk_cast(nm, m, sc, ps)
                    return emit
                return [(deadline - (3 - d), q(d)) for d in range(4)]

            def v_emit(hp, jc):
                """v projection for head-pair hp, j-chunk jc (prologue)."""
                ps = pp.tile([128, 512], F32, tag="sc", bufs=2, name="psv")
                for d in range(4):
                    nc.tensor.matmul(
                        ps[:, 0:128], xts(d, jc * 128, (jc + 1) * 128),
                        wv_v(d, hp),
                        start=(d == 0), stop=(d == 3),
                        skip_group_check=True)
                base = jc * HPC * VW + hp * 2 * VW
                out_view = vv[:, base:base + 2 * VW].rearrange(
                    "p (h w) -> p h w", w=VW)[:, :, HD:VW]
                nc.vector.tensor_copy(
                    out=out_view,
                    in_=ps[:, 0:128].rearrange("p (h d) -> p h d", d=HD))
                vv_done.add((hp, jc))

            def v_quanta(hp, jc, deadline):
                """same, split into two 2-matmul quanta for smoothing."""
                state = {}
                def q(g):
                    def emit():
                        if g == 0:
                            state["ps"] = pp.tile([128, 512], F32, tag="sc",
                                                  bufs=2, name="psv")
                        ps = state["ps"]
                        for d in (2 * g, 2 * g + 1):
                            nc.tensor.matmul(
                                ps[:, 0:128], xts(d, jc * 128, (jc + 1) * 128),
                                wv_v(d, hp),
                                start=(d == 0), stop=(d == 3),
                                skip_group_check=True)
                        if g == 1:
                            base = jc * HPC * VW + hp * 2 * VW
                            out_view = vv[:, base:base + 2 * VW].rearrange(
                                "p (h w) -> p h w", w=VW)[:, :, HD:VW]
                            nc.vector.tensor_copy(
                                out=out_view,
                                in_=ps[:, 0:128].rearrange(
                                    "p (h d) -> p h d", d=HD))
                            vv_done.add((hp, jc))
                    return emit
                return [(deadline - 1, q(0)), (deadline, q(1))]

            # per-scq gather buffers for the output projection: the 4
            # m-groups of one scq cast into one [128, 2048] tile, then ONE
            # dma_start ships all 512KB (one Sync trigger instead of 4)
            ob_state = {}
            # emission-order guards: the tile tracker links readers only to
            # PRIOR writes, so a consumer emitted before its producer is a
            # silent race. Record what has been emitted; assert before use.
            vv_done = set()
            qk_done = set()

            def out_quanta(m, scq, deadline, pool_tag="sc", cast_eng="dve"):
                """output projection group: 2 matmul quanta + cast (+dma
                after the scq's 4th group)."""
                state = {}
                def q(kc):
                    def emit():
                        if kc == 0:
                            if pool_tag == "sp":
                                state["ps"] = pp.tile([128, IH], F32,
                                                      tag="sp", bufs=2,
                                                      name="psot")
                            else:
                                state["ps"] = pp.tile([128, 512], F32,
                                                      tag="sc", bufs=2,
                                                      name="pso")
                        ps = state["ps"]
                        nc.tensor.matmul(
                            ps[:, 0:512], wo_v(kc, m),
                            oTn[kc][:, scq * 512:(scq + 1) * 512],
                            start=(kc == 0), stop=(kc == 1),
                            skip_group_check=True)
                        if kc == 1:
                            if scq not in ob_state:
                                ob_state[scq] = [sb.tile(
                                    [128, 2048], F16, tag="ob", bufs=2,
                                    name=f"ob{scq}"), 0]
                            obt, _ = ob_state[scq]
                            dst = obt[:, m * 512:(m + 1) * 512]
                            if cast_eng == "act":
                                nc.scalar.activation(
                                    dst, ps[:, 0:512],
                                    mybir.ActivationFunctionType.Copy)
                            else:
                                nc.vector.tensor_copy(
                                    out=dst, in_=ps[:, 0:512])
                            ob_state[scq][1] += 1
                            # ship in m-halves so the first 256KB is in
                            # flight while the second half's casts run
                            if ob_state[scq][1] in (2, 4):
                                mh = ob_state[scq][1] // 2 - 1  # 0 or 1
                                nc.sync.dma_start(
                                    out=outT[:, :].rearrange(
                                        "p (m s) -> p m s", s=2048)[
                                        :, 2 * mh:2 * mh + 2,
                                        scq * 512:(scq + 1) * 512],
                                    in_=obt[:, mh * 1024:(mh + 1) * 1024]
                                    .rearrange("p (m s) -> p m s", s=512))
                    return emit
                return [(deadline, q(0)), (deadline, q(1))]

            # ---- prologue projections: everything unit (0,0) needs that
            # only depends on the first two x column-quarters ----
            with nc.named_scope("proj"):
                qk_group("k", 0, 0)
                qk_group("q", 0, 0)
                qk_group("q", 0, 1)
                for jc in range(3):
                    v_emit(0, jc)

            # ---- deferred-work queue: (deadline_slot, emit) sorted ----
            defq = []
            defq += qk_quanta("k", 0, 1, 4)      # scores(0,0) jc>=4
            defq += qk_quanta("k", 0, 2, 8)      # scores(0,0) jc>=8
            defq += qk_quanta("k", 0, 3, 12)
            # AV(u, jc) normally pops at slot jc+AV_LAG, but the unit
            # boundary drains the last 4 AVs early (2/slot at jc 0-1), so
            # clamp the tail deadlines below that
            for jc in range(3, 16):
                defq += v_quanta(0, jc, jc + AV_LAG if jc < 12 else jc - 1)
            defq += qk_quanta("q", 1, 0, 24)     # unit (2,0) scores @ slot 31
            defq += qk_quanta("q", 1, 1, 28)
            defq += qk_quanta("k", 1, 0, 29)
            defq += qk_quanta("k", 1, 1, 35)
            defq += qk_quanta("k", 1, 2, 39)
            defq += qk_quanta("k", 1, 3, 43)
            for jc in range(16):
                defq += v_quanta(1, jc,
                                 32 + (jc + AV_LAG if jc < 12 else jc - 1))
            defq += qk_quanta("q", 0, 2, 57)     # unit (0,1) scores @ slot 63
            defq += qk_quanta("q", 0, 3, 61)
            defq += qk_quanta("q", 1, 2, 89)     # unit (2,1) scores @ slot 95
            defq += qk_quanta("q", 1, 3, 93)
            defq.sort(key=lambda t: t[0])
            outproj_v0 = []   # gated on epilogue of unit 3 (~slot 67)
            for scq in range(2):      # scq-major so the combined per-scq
                for m in range(4):    # DMA fires right after its 4th group
                    outproj_v0 += out_quanta(m, scq, 120)
            outproj_v1 = []   # tail: needs the last unit's epilogue.
            # scq-major: the first half only reads columns the first
            # epilogue-half has normalized; casts alternate ACT/DVE (both
            # idle in the tail) so the cast chain halves
            gi = 0
            for scq in range(2, 4):
                for m in range(4):
                    outproj_v1 += out_quanta(
                        m, scq, 999, pool_tag=("sp" if gi % 2 else "sc"),
                        cast_eng=("act" if m % 2 == 0 else "dve"))
                    gi += 1

            # ---- attention: units (h, v) v-major; software pipeline ----
            units = [(h, v) for v in range(2) for h in range(4)]
            nunits = len(units)

            def epilogue(uid, op, c0=0, c1=IH):
                """normalize straight out of op psum: rows 0..63 hold the
                softmax denominator replicated via the ones-columns of vv,
                rows 64..127 the unnormalized output. The next unit's AV
                start=True write WAR-waits on these two reads."""
                h, v = units[uid]
                p, off = h // 2, 64 * (h % 2)
                recip = sb.tile([64, IH], F32, tag="recip", bufs=2,
                                name="recip")
                nc.vector.reciprocal_approx_fast(
                    out=recip[:, c0:c1], in_=op[0:64, c0:c1])
                nc.vector.tensor_mul(
                    out=oTn[p][off:off + 64, v * IH + c0:v * IH + c1],
                    in0=op[64:128, c0:c1], in1=recip[:, c0:c1])

            with nc.named_scope("attn"):
                pend_av = []       # (uid, h, jc, at_tile, op_tile-or-None)
                op_t = [None]      # current unit's op accumulator
                op_prev = [None]
                sp_pend = {}       # s -> sp tile (scores emitted, exp not)

                def emit_scores(s):
                    """scores [j(128), i(1024)] for global slot s. Emitted
                    one slot AHEAD of its exp so the exp never waits on the
                    scores-completion semaphore (~150ns/slot otherwise)."""
                    uu, jj = s // 16, s % 16
                    hh, vv_ = units[uu]
                    mm, oo, ii0 = hh // 2, 64 * (hh % 2), vv_ * IH
                    assert ("k", mm, jj // 4) in qk_done, ("k", s, mm, jj)
                    sp = pp.tile([128, IH], F32, tag="sp", bufs=2,
                                 name="sp")
                    for scc in range(2):
                        assert ("q", mm, (ii0 + scc * 512) // 512) in qk_done, \
                            ("q", s, mm, ii0, scc)
                        nc.tensor.matmul(
                            sp[:, scc * 512:(scc + 1) * 512],
                            kT[mm][oo:oo + 64, jj * 128:(jj + 1) * 128],
                            qT[mm][oo:oo + 64,
                                   ii0 + scc * 512:ii0 + (scc + 1) * 512],
                            start=True, stop=True)
                    sp_pend[s] = sp

                def emit_av():
                    _, hh, jj, aa, oo = pend_av.pop(0)
                    if oo is None:
                        oo = op_t[0]
                    assert (hh // 2, jj) in vv_done, ("vv", hh, jj)
                    base = jj * HPC * VW + hh * VW
                    for scc in range(2):
                        nc.tensor.matmul(
                            oo[:, scc * 512:(scc + 1) * 512],
                            vv[:, base:base + VW],
                            aa[:, scc * 512:(scc + 1) * 512],
                            start=(jj == 0), stop=(jj == 15),
                            skip_group_check=True)

                emit_scores(0)
                for ui, (h, v) in enumerate(units):
                    for jc in range(16):
                        s = ui * 16 + jc
                        at_t = sb.tile([128, IH], BF16, tag="at", bufs=12,
                                       name="at")
                        nc.scalar.activation(at_t[:], sp_pend.pop(s), EXP)
                        if s + 1 < 16 * nunits:
                            emit_scores(s + 1)
                        # acquire op right before this unit's first AV (and
                        # after the previous unit's epilogue was emitted)
                        if jc == AV_LAG:
                            op_t[0] = pp.tile([128, IH], F32, tag="op",
                                              bufs=1, name="op")
                        pend_av.append(
                            (ui, h, jc, at_t,
                             op_t[0] if jc >= AV_LAG else None))
                        # deferred proj/outproj quanta in the PE slack.
                        # Boundary slots jc 0-1 carry 2 AV drains each (PE
                        # ~1.6us, over budget) -> no pops there; jc 2-3
                        # carry no AV at all (~540ns slack) -> up to 3 pops.
                        # Net over the 4 boundary slots the PE load is flat.
                        popped = 0
                        cap = 1
                        if ui > 0 and jc < 2:
                            cap = 0
                        elif ui > 0 and jc in (2, 3):
                            cap = 3
                        while defq and (popped < cap
                                        or defq[0][0] <= s + (2 if cap else 0)):
                            defq.pop(0)[1]()
                            popped += 1
                        if not defq and outproj_v0 and s >= 70:
                            want = min(cap - popped, len(outproj_v0))
                            for _ in range(max(0, want)):
                                outproj_v0.pop(0)[1]()
                        if len(pend_av) > AV_LAG:
                            emit_av()
                        # boundary: drain the previous unit's AVs 2/slot,
                        # then emit its epilogue right after its last AV so
                        # the op psum is free before this unit's first AV
                        # (popped at jc==AV_LAG) needs the banks
                        if (ui > 0 and jc < 2 and pend_av
                                and pend_av[0][0] == ui - 1):
                            emit_av()
                            if jc == 1:
                                while pend_av and pend_av[0][0] == ui - 1:
                                    emit_av()
                                epilogue(ui - 1, op_prev[0])
                        # last unit: pre-drain so the tail chain is short
                        if (ui == nunits - 1 and jc >= 12
                                and len(pend_av) > 1):
                            emit_av()
                    op_prev[0] = op_t[0]

                # drain the final unit's pipeline; both epilogue halves
                # first so the DVE normalization chain runs back-to-back
                # (nothing else needs the op psum), then the v0 leftovers
                # keep the PE hot until the v1 output projection unblocks
                while pend_av:
                    emit_av()
                last = nunits - 1
                epilogue(last, op_prev[0], 0, 512)
                epilogue(last, op_prev[0], 512, IH)
                for _, q in outproj_v0:
                    q()

            # ---- tail: remaining output projection ----
            with nc.named_scope("outproj"):
                for _, q in outproj_v1:
                    q()

    nc.compile()
    return nc


def _get_nc():
    if "nc" not in _cache:
        _cache["nc"] = _build_nc()
    return _cache["nc"]


def _fold(a, nblk):
    """[nblk*128, C] -> [128, nblk*C] with d-major column blocks."""
    r, c = a.shape
    assert r == nblk * 128
    return np.ascontiguousarray(
        a.reshape(nblk, 128, c).transpose(1, 0, 2).reshape(128, nblk * c))


def _in_maps(x, w_qkv, w_out):
    x = np.asarray(x, dtype=np.float32)
    w_qkv = np.asarray(w_qkv, dtype=np.float32)
    w_out = np.asarray(w_out, dtype=np.float32)
    maps = []
    for c in range(NCORES):
        b, qh = c // 2, c % 2
        r0 = qh * DQ

        def fold_m(a, m):  # [512, 256] -> [128, 512] (d-major, head-pair m)
            return np.concatenate(
                [a[128 * d:128 * (d + 1), 128 * m:128 * (m + 1)]
                 for d in range(4)], axis=1)

        wk = w_qkv[D + r0:D + r0 + DQ].T                # [512, 256]
        wq = w_qkv[r0:r0 + DQ].T
        wv = _fold(w_qkv[2 * D + r0:2 * D + r0 + DQ].T, 4)
        wo = _fold(w_out[:, r0:r0 + DQ].T, 2)           # [128, 1024]
        xT = x[b].T                                     # [512, 2048]
        maps.append({
            "wkqAT": np.concatenate(
                [fold_m(wk, 0), fold_m(wq, 0)], axis=1).astype(np.float16),
            "wkqBT": np.concatenate(
                [fold_m(wk, 1), fold_m(wq, 1)], axis=1).astype(np.float16),
            "wvT": wv.astype(np.float16),
            "woT": wo.astype(np.float16),
            **{f"xQ{r}": _fold(
                xT[:, r * 512:(r + 1) * 512], 4).astype(np.float16)
               for r in range(4)},
        })
    return maps


def _gather(results):
    out = np.empty((B, S, D), np.float32)
    for b in range(B):
        acc = np.zeros((512, 2048), np.float32)
        for c in (2 * b, 2 * b + 1):
            o = results[c]["outT"].astype(np.float32)   # [128, 8192]
            acc += o.reshape(128, 4, 2048).transpose(1, 0, 2).reshape(
                512, 2048)
        out[b] = acc.T
    return out


def run(x, w_qkv, w_out, trace=False):
    from concourse.bass_utils import run_bass_kernel_spmd

    nc = _get_nc()
    res = run_bass_kernel_spmd(
        nc, _in_maps(x, w_qkv, w_out), core_ids=list(range(NCORES)), trace=trace,
    )
    return _gather(res.results), res


def kernel(x, w_qkv, w_out):
    out, _ = run(x, w_qkv, w_out)
    return out


# revision 26
# speedup vs baseline: 1.1259x; 1.1259x over previous
"""Multi-head attention (B=4, S=2048, D=512, H=8) on 8 trn2 cores.

Sharding: core c handles batch b=c//2 and the head-quad qh=c%2 (heads
4*qh..4*qh+3). Each core computes q/k/v projections for its 4 heads over the
full sequence, flash-style attention (scores kept transposed [j, i] so all
matmul contractions land on the partition dim with zero on-device transposes),
and the partial output projection over its 256 o-dims. The host pre-packs
x/weight slices into sbuf-layout 2D dram tensors (free) and sums/transposes
the two partial outputs per batch.

Design (single fused pipeline; HW trace shows ACT is the steady-state
metronome at ~1.08us/slot with PE just underneath it):
 - The scalar engine's exp is the hard floor: 128 exp tiles of [128,1024]
   ~= 139us/core busy. The schedule keeps ACT saturated: everything else
   (projections, output projection, normalization) hides in PE/DVE slack.
 - Attention inner loop is software-pipelined with the PE stream ordered
   [scores(jc), AV(jc-3), deferred-quantum]; at bufs=10 decouples the exp
   WAR from AV jitter at unit boundaries.
 - Input DMAs: host packs weights and x into contiguous [128, N] dram
   tensors matching sbuf layout exactly, so the whole input stream is 7
   triggers with 4-8KB descriptors, ordered by first use (wkq -> x quarters
   -> wv -> wo). First exp fires ~9us in instead of ~24us.
 - The q/k/v projections and the output projection are cut into quanta and
   drip-fed into the attention loop's PE slack from a deadline-sorted queue
   with staggered deadlines (~1 quantum/slot, no bursts). PSUM: sp
   [128,1024]x2 + op [128,1024]x1 + scratch [128,512]x2 = exactly 8 banks.
 - Softmax normalization without DRAM round-trips: each v block carries 64
   ones-columns ([128,128] stationary = 64 ones | 64 v), so the AV matmul
   replicates the softmax denominator into op psum rows 0..63 for free. The
   epilogue reads op PSUM directly: reciprocal_approx_fast (base-0
   partitions, 18-bit exact; sums are ~[1,1e20], far from its denorm/inf
   edge cases) then one multiply. No drain copies - the next unit's AV
   start=True write WAR-waits on these two reads, same latency as the old
   copy pair but 2 fewer DVE ops per unit.
 - Unit boundaries: previous unit's last AVs drain 2/slot at jc 0-1, its
   epilogue is emitted at jc==2 (one slot before the op psum is re-acquired
   at jc==3), shrinking the boundary stall.
 - fp16 for the score path (x, w_qkv, q, k, w_out, o): 1 cycle/row on the PE
   like bf16 but 8x the mantissa (bf16 q/k fails the 2e-2 gate at ~2.2e-2;
   fp16 lands at 3.3e-3). exp output (attn weights) stays bf16 for fp32
   exponent range since softmax skips max-subtraction, psum stays fp32.
   Output partials are written fp16 (host sums in fp32): halves the output
   DMA and the tail transfer.
"""
import sys

sys.path.insert(0, "/opt/trn_rl_repo")
import numpy as np

B, S, D, H, HD = 4, 2048, 512, 8, 64
HPC = 4          # heads per core
DQ = HPC * HD    # 256 projection dims per core
NCORES = 8
VW = 2 * HD      # v block width: 64 v-dims + 64 ones columns (128)
IH = S // 2      # i-half processed per attention unit (1024)
AV_LAG = 4       # attn@v trails scores by this many j-chunks

_cache = {}


def _build_nc():
    import concourse.bacc as bacc
    import concourse.mybir as mybir
    import concourse.tile as tile

    F32 = mybir.dt.float32
    F16, BF16 = mybir.dt.float16, mybir.dt.bfloat16
    EXP = mybir.ActivationFunctionType.Exp

    nc = bacc.Bacc("TRN2", target_bir_lowering=False, debug=False)

    # host-packed inputs, all contiguous 2D [128, N] in final sbuf layout:
    # wkqA/B: [k d-blocks (4x128) | q d-blocks (4x128)] for head-pair m=0/1
    # wvT: [v d-blocks (4x256)]
    # woT: [wo kc-blocks (2x512)]
    # xQ0..3: x column-quarters, d-major blocks of 512 cols each
    wkqAT = nc.dram_tensor("wkqAT", [128, 1024], F16, kind="ExternalInput")
    wkqBT = nc.dram_tensor("wkqBT", [128, 1024], F16, kind="ExternalInput")
    wvT = nc.dram_tensor("wvT", [128, 1024], F16, kind="ExternalInput")
    woT = nc.dram_tensor("woT", [128, 1024], F16, kind="ExternalInput")
    xQ = [nc.dram_tensor(f"xQ{r}", [128, 2048], F16, kind="ExternalInput")
          for r in range(4)]
    # output: block m (cols m*2048+s) holds out-dims m*128..m*128+127
    outT = nc.dram_tensor("outT", [128, 8192], F16, kind="ExternalOutput")

    with tile.TileContext(nc) as tc:
        with tc.tile_pool(name="sb", bufs=1) as sb, \
             tc.tile_pool(name="ps", bufs=1, space="PSUM") as pp:
            # ---- persistent sbuf tensors ----
            wkq = [sb.tile([128, 1024], F16, tag=f"wkq{m}", name=f"wkq{m}")
                   for m in range(2)]
            wvs = sb.tile([128, 1024], F16, tag="wvs", name="wvs")
            wos = sb.tile([128, 1024], F16, tag="wos", name="wos")
            xtf = sb.tile([128, 8192], F16, tag="xtf", name="xtf")

            def wk_v(d, m):      # k weights, d-block, head-pair m
                return wkq[m][:, d * 128:(d + 1) * 128]

            def wq_v(d, m):
                return wkq[m][:, 512 + d * 128:512 + (d + 1) * 128]

            def wv_v(d, hp):     # v weights, d-block, head-pair hp
                return wvs[:, d * 256 + hp * 128:d * 256 + (hp + 1) * 128]

            def wo_v(kc, m):
                return wos[:, kc * 512 + m * 128:kc * 512 + (m + 1) * 128]

            def xts(d, c0, c1):  # x cols c0:c1 of d-block (one 512-region)
                r = c0 // 512
                assert (c1 - 1) // 512 == r
                base = r * 2048 + d * 512 + (c0 - r * 512)
                return xtf[:, base:base + (c1 - c0)]

            # ---- input DMAs ----
            # The 16 DMA engines service all queued transfers round-robin by
            # descriptor, so every queued dma_start completes only near the
            # end of the whole in-flight stream. Free-run only what the
            # prologue needs (wkqA, xQ0, wv, xQ1 = 1.5MB); gate the rest
            # behind an xQ1-completion token (tiny GPSIMD copies into each
            # gated dst chain the gated DMA's WAR dependency to xQ1). A
            # deeper 3-level chain measured FASTER on the best core but blew
            # up tail-core variance (late levels miss defq deadlines), so
            # stay at one level.
            nc.sync.dma_start(out=wkq[0][:], in_=wkqAT[:, :])
            nc.sync.dma_start(out=xtf[:, 0:2048], in_=xQ[0][:, :])
            nc.sync.dma_start(out=wvs[:], in_=wvT[:, :])
            nc.sync.dma_start(out=xtf[:, 2048:4096], in_=xQ[1][:, :])
            tok1 = xtf[0:1, 2048:2049]    # written by xQ1
            for dst in (xtf[0:1, 4096:4097], xtf[0:1, 6144:6145],
                        wkq[1][0:1, 0:1], wos[0:1, 0:1]):
                nc.gpsimd.tensor_copy(out=dst, in_=tok1)
            nc.sync.dma_start(out=xtf[:, 4096:6144], in_=xQ[2][:, :])
            nc.sync.dma_start(out=xtf[:, 6144:8192], in_=xQ[3][:, :])
            nc.sync.dma_start(out=wkq[1][:], in_=wkqBT[:, :])
            nc.sync.dma_start(out=wos[:], in_=woT[:, :])

            qT = [sb.tile([128, S], F16, tag=f"qT{m}", name=f"qT{m}")
                  for m in range(2)]
            kT = [sb.tile([128, S], F16, tag=f"kT{m}", name=f"kT{m}")
                  for m in range(2)]
            # vv block for (jc, h): cols [0:64] = ones, [64:128] = v dims
            # (ones first so the softmax sums land at psum partitions 0:64,
            # where the base-0 custom-DVE fast reciprocal can read them)
            vv = sb.tile([128, 16 * HPC * VW], BF16, tag="vv", name="vv")
            # oTn[p]: heads (2p, 2p+1) stacked on partitions; outproj moving
            oTn = [sb.tile([128, S], F16, tag=f"oTn{p}", name=f"oTn{p}")
                   for p in range(2)]

            # ---- prologue scratch: ACT table preload + PE warm-up ----
            wub = sb.tile([128, 512], BF16, tag="wub", name="wub")
            nc.vector.memset(wub[:], 0.25)
            # tiny exp: forces the ACT exp table load off the critical path
            dummy_at = sb.tile([128, 16], BF16, tag="dummy_at", name="dummy_at")
            nc.scalar.activation(dummy_at[:], wub[:, 0:16], EXP)
            ones32 = sb.tile([128, 1], F32, tag="ones32", name="ones32")
            nc.vector.memset(ones32[:], 1.0)
            vv_ones = vv[:, :].rearrange("p (g w) -> p g w", w=VW)[:, :, 0:HD]
            nc.vector.tensor_copy(
                out=vv_ones, in_=ones32[:].to_broadcast((128, 16 * HPC, HD)))
            # keep the PE busy/ramping (HAM warm) while the critical input
            # DMAs stream in; 256-wide quanta so the tail of the warm-up
            # doesn't delay the first projection once data lands
            for _ in range(16):
                wups = pp.tile([128, 512], F32, tag="sc", bufs=2, name="wups")
                nc.tensor.matmul(wups[:, 0:256], wub[:, 0:128],
                                 wub[:, 0:256],
                                 start=True, stop=True, skip_group_check=True)

            # ---- work-group emitters ----
            def qk_cast(nm, m, sc, ps):
                tgt = (qT if nm == "q" else kT)[m]
                nc.vector.tensor_copy(
                    out=tgt[:, sc * 512:(sc + 1) * 512], in_=ps[:, 0:512])
                qk_done.add((nm, m, sc))

            def qk_group(nm, m, sc):
                """whole q/k projection group: 4 matmuls + cast (prologue)."""
                ps = pp.tile([128, 512], F32, tag="sc", bufs=2, name="ps")
                wf = wq_v if nm == "q" else wk_v
                for d in range(4):
                    nc.tensor.matmul(
                        ps[:, 0:512], wf(d, m),
                        xts(d, sc * 512, (sc + 1) * 512),
                        start=(d == 0), stop=(d == 3))
                qk_cast(nm, m, sc, ps)

            def qk_quanta(nm, m, sc, deadline):
                """same group cut into 4 one-matmul quanta for the defq,
                with staggered deadlines so pops stay ~1/slot."""
                state = {}
                def q(d):
                    def emit():
                        if d == 0:
                            state["ps"] = pp.tile([128, 512], F32, tag="sc",
                                                  bufs=2, name="psq")
                        ps = state["ps"]
                        wf = wq_v if nm == "q" else wk_v
                        nc.tensor.matmul(
                            ps[:, 0:512], wf(d, m),
                            xts(d, sc * 512, (sc + 1) * 512),
                            start=(d == 0), stop=(d == 3),
                            skip_group_check=True)
                        if d == 3:
                            qk_cast(nm, m, sc, ps)
                    return emit
                return [(deadline - (3 - d), q(d)) for d in range(4)]

            def v_emit(hp, jc):
                """v projection for head-pair hp, j-chunk jc (prologue)."""
                ps = pp.tile([128, 512], F32, tag="sc", bufs=2, name="psv")
                for d in range(4):
                    nc.tensor.matmul(
                        ps[:, 0:128], xts(d, jc * 128, (jc + 1) * 128),
                        wv_v(d, hp),
                        start=(d == 0), stop=(d == 3),
                        skip_group_check=True)
                base = jc * HPC * VW + hp * 2 * VW
                out_view = vv[:, base:base + 2 * VW].rearrange(
                    "p (h w) -> p h w", w=VW)[:, :, HD:VW]
                nc.vector.tensor_copy(
                    out=out_view,
                    in_=ps[:, 0:128].rearrange("p (h d) -> p h d", d=HD))
                vv_done.add((hp, jc))

            def v_quanta(hp, jc, deadline):
                """same, split into two 2-matmul quanta for smoothing."""
                state = {}
                def q(g):
                    def emit():
                        if g == 0:
                            state["ps"] = pp.tile([128, 512], F32, tag="sc",
                                                  bufs=2, name="psv")
                        ps = state["ps"]
                        for d in (2 * g, 2 * g + 1):
                            nc.tensor.matmul(
                                ps[:, 0:128], xts(d, jc * 128, (jc + 1) * 128),
                                wv_v(d, hp),
                                start=(d == 0), stop=(d == 3),
                                skip_group_check=True)
                        if g == 1:
                            base = jc * HPC * VW + hp * 2 * VW
                            out_view = vv[:, base:base + 2 * VW].rearrange(
                                "p (h w) -> p h w", w=VW)[:, :, HD:VW]
                            nc.vector.tensor_copy(
                                out=out_view,
                                in_=ps[:, 0:128].rearrange(
                                    "p (h d) -> p h d", d=HD))
                            vv_done.add((hp, jc))
                    return emit
                return [(deadline - 1, q(0)), (deadline, q(1))]

            # per-scq gather buffers for the output projection: the 4
            # m-groups of one scq cast into one [128, 2048] tile, then ONE
            # dma_start ships all 512KB (one Sync trigger instead of 4)
            ob_state = {}
            # emission-order guards: the tile tracker links readers only to
            # PRIOR writes, so a consumer emitted before its producer is a
            # silent race. Record what has been emitted; assert before use.
            vv_done = set()
            qk_done = set()

            def out_quanta(m, scq, deadline, pool_tag="sc", cast_eng="dve"):
                """output projection group: 2 matmul quanta + cast (+dma
                after the scq's 4th group)."""
                state = {}
                def q(kc):
                    def emit():
                        if kc == 0:
                            if pool_tag == "sp":
                                state["ps"] = pp.tile([128, IH], F32,
                                                      tag="sp", bufs=2,
                                                      name="psot")
                            else:
                                state["ps"] = pp.tile([128, 512], F32,
                                                      tag="sc", bufs=2,
                                                      name="pso")
                        ps = state["ps"]
                        nc.tensor.matmul(
                            ps[:, 0:512], wo_v(kc, m),
                            oTn[kc][:, scq * 512:(scq + 1) * 512],
                            start=(kc == 0), stop=(kc == 1),
                            skip_group_check=True)
                        if kc == 1:
                            if scq not in ob_state:
                                ob_state[scq] = [sb.tile(
                                    [128, 2048], F16, tag="ob", bufs=2,
                                    name=f"ob{scq}"), 0]
                            obt, _ = ob_state[scq]
                            dst = obt[:, m * 512:(m + 1) * 512]
                            if cast_eng == "act":
                                nc.scalar.activation(
                                    dst, ps[:, 0:512],
                                    mybir.ActivationFunctionType.Copy)
                            else:
                                nc.vector.tensor_copy(
                                    out=dst, in_=ps[:, 0:512])
                            ob_state[scq][1] += 1
                            # ship in m-halves so the first 256KB is in
                            # flight while the second half's casts run
                            if ob_state[scq][1] in (2, 4):
                                mh = ob_state[scq][1] // 2 - 1  # 0 or 1
                                nc.sync.dma_start(
                                    out=outT[:, :].rearrange(
                                        "p (m s) -> p m s", s=2048)[
                                        :, 2 * mh:2 * mh + 2,
                                        scq * 512:(scq + 1) * 512],
                                    in_=obt[:, mh * 1024:(mh + 1) * 1024]
                                    .rearrange("p (m s) -> p m s", s=512))
                    return emit
                return [(deadline, q(0)), (deadline, q(1))]

            # ---- prologue projections: everything unit (0,0) needs that
            # only depends on the first two x column-quarters ----
            with nc.named_scope("proj"):
                qk_group("k", 0, 0)
                qk_group("q", 0, 0)
                qk_group("q", 0, 1)
                for jc in range(3):
                    v_emit(0, jc)

            # ---- deferred-work queue: (deadline_slot, emit) sorted ----
            defq = []
            defq += qk_quanta("k", 0, 1, 4)      # scores(0,0) jc>=4
            defq += qk_quanta("k", 0, 2, 8)      # scores(0,0) jc>=8
            defq += qk_quanta("k", 0, 3, 12)
            # AV(u, jc) normally pops at slot jc+AV_LAG, but the unit
            # boundary drains the last 4 AVs early (2/slot at jc 0-1), so
            # clamp the tail deadlines below that
            for jc in range(3, 16):
                defq += v_quanta(0, jc, jc + AV_LAG if jc < 12 else jc - 1)
            # unit order defers all m=1 tensors to slot 64+, so the deferred
            # work spreads at <=1.5 pops/slot instead of 2.4 in units 0-2:
            #   units: h0v0 h1v0 | h0v1 h1v1 | h2v0 h3v0 | h2v1 h3v1
            defq += qk_quanta("q", 0, 2, 27)     # unit 2 scores @ slot 31
            defq += qk_quanta("q", 0, 3, 31)
            for jc in range(16):                 # vv hp1 for unit 4 @ 64+
                defq += v_quanta(1, jc, 34 + (27 * jc) // 16)
            defq += qk_quanta("q", 1, 0, 56)     # unit 4 scores @ slot 63
            defq += qk_quanta("q", 1, 1, 60)
            defq += qk_quanta("k", 1, 0, 61)
            defq += qk_quanta("k", 1, 1, 64)
            defq += qk_quanta("k", 1, 2, 68)
            defq += qk_quanta("k", 1, 3, 72)
            defq += qk_quanta("q", 1, 2, 88)     # unit 6 scores @ slot 95
            defq += qk_quanta("q", 1, 3, 92)
            defq.sort(key=lambda t: t[0])
            outproj_v0 = []   # gated on epilogue of unit 5 (h3,v0) ~slot 97
            for scq in range(2):      # scq-major so the combined per-scq
                for m in range(4):    # DMA fires right after its 4th group
                    outproj_v0 += out_quanta(m, scq, 120)
            outproj_v1 = []   # tail: needs the last unit's epilogue.
            # scq-major: the first half only reads columns the first
            # epilogue-half has normalized; casts alternate ACT/DVE (both
            # idle in the tail) so the cast chain halves
            gi = 0
            for scq in range(2, 4):
                for m in range(4):
                    outproj_v1 += out_quanta(
                        m, scq, 999, pool_tag=("sp" if gi % 2 else "sc"),
                        cast_eng=("act" if m % 2 == 0 else "dve"))
                    gi += 1

            # ---- attention: unit order keeps head-pair 0 first (defers the
            # m=1 projections), v0 before v1 within each pair so outproj
            # halves unlock as early as possible ----
            units = [(0, 0), (1, 0), (0, 1), (1, 1),
                     (2, 0), (3, 0), (2, 1), (3, 1)]
            nunits = len(units)

            def epilogue(uid, op, c0=0, c1=IH):
                """normalize straight out of op psum: rows 0..63 hold the
                softmax denominator replicated via the ones-columns of vv,
                rows 64..127 the unnormalized output. The next unit's AV
                start=True write WAR-waits on these two reads."""
                h, v = units[uid]
                p, off = h // 2, 64 * (h % 2)
                recip = sb.tile([64, IH], F32, tag="recip", bufs=2,
                                name="recip")
                nc.vector.reciprocal_approx_fast(
                    out=recip[:, c0:c1], in_=op[0:64, c0:c1])
                nc.vector.tensor_mul(
                    out=oTn[p][off:off + 64, v * IH + c0:v * IH + c1],
                    in0=op[64:128, c0:c1], in1=recip[:, c0:c1])

            with nc.named_scope("attn"):
                pend_av = []       # (uid, h, jc, at_tile, op_tile-or-None)
                op_t = [None]      # current unit's op accumulator
                op_prev = [None]
                sp_pend = {}       # s -> sp tile (scores emitted, exp not)

                def emit_scores(s):
                    """scores [j(128), i(1024)] for global slot s. Emitted
                    one slot AHEAD of its exp so the exp never waits on the
                    scores-completion semaphore (~150ns/slot otherwise)."""
                    uu, jj = s // 16, s % 16
                    hh, vv_ = units[uu]
                    mm, oo, ii0 = hh // 2, 64 * (hh % 2), vv_ * IH
                    assert ("k", mm, jj // 4) in qk_done, ("k", s, mm, jj)
                    sp = pp.tile([128, IH], F32, tag="sp", bufs=2,
                                 name="sp")
                    for scc in range(2):
                        assert ("q", mm, (ii0 + scc * 512) // 512) in qk_done, \
                            ("q", s, mm, ii0, scc)
                        nc.tensor.matmul(
                            sp[:, scc * 512:(scc + 1) * 512],
                            kT[mm][oo:oo + 64, jj * 128:(jj + 1) * 128],
                            qT[mm][oo:oo + 64,
                                   ii0 + scc * 512:ii0 + (scc + 1) * 512],
                            start=True, stop=True)
                    sp_pend[s] = sp

                def emit_av():
                    _, hh, jj, aa, oo = pend_av.pop(0)
                    if oo is None:
                        oo = op_t[0]
                    assert (hh // 2, jj) in vv_done, ("vv", hh, jj)
                    base = jj * HPC * VW + hh * VW
                    for scc in range(2):
                        nc.tensor.matmul(
                            oo[:, scc * 512:(scc + 1) * 512],
                            vv[:, base:base + VW],
                            aa[:, scc * 512:(scc + 1) * 512],
                            start=(jj == 0), stop=(jj == 15),
                            skip_group_check=True)

                emit_scores(0)
                for ui, (h, v) in enumerate(units):
                    for jc in range(16):
                        s = ui * 16 + jc
                        at_t = sb.tile([128, IH], BF16, tag="at", bufs=12,
                                       name="at")
                        nc.scalar.activation(at_t[:], sp_pend.pop(s), EXP)
                        if s + 1 < 16 * nunits:
                            emit_scores(s + 1)
                        # acquire op right before this unit's first AV (and
                        # after the previous unit's epilogue was emitted)
                        if jc == AV_LAG:
                            op_t[0] = pp.tile([128, IH], F32, tag="op",
                                              bufs=1, name="op")
                        pend_av.append(
                            (ui, h, jc, at_t,
                             op_t[0] if jc >= AV_LAG else None))
                        # deferred proj/outproj quanta in the PE slack.
                        # Boundary slots jc 0-1 carry 2 AV drains each (PE
                        # ~1.6us, over budget) -> no pops there; jc 2-3
                        # carry no AV at all (~540ns slack) -> up to 3 pops.
                        # Net over the 4 boundary slots the PE load is flat.
                        popped = 0
                        cap = 1
                        if ui > 0 and jc < 2:
                            cap = 0
                        elif ui > 0 and jc in (2, 3):
                            cap = 3
                        while defq and (popped < cap
                                        or defq[0][0] <= s + (2 if cap else 0)):
                            defq.pop(0)[1]()
                            popped += 1
                        if not defq and outproj_v0 and s >= 99:
                            want = min(cap - popped, len(outproj_v0))
                            for _ in range(max(0, want)):
                                outproj_v0.pop(0)[1]()
                        if len(pend_av) > AV_LAG:
                            emit_av()
                        # boundary: drain the previous unit's AVs 2/slot,
                        # then emit its epilogue right after its last AV so
                        # the op psum is free before this unit's first AV
                        # (popped at jc==AV_LAG) needs the banks
                        if (ui > 0 and jc < 2 and pend_av
                                and pend_av[0][0] == ui - 1):
                            emit_av()
                            if jc == 1:
                                while pend_av and pend_av[0][0] == ui - 1:
                                    emit_av()
                                epilogue(ui - 1, op_prev[0])
                        # last unit: pre-drain so the tail chain is short
                        if (ui == nunits - 1 and jc >= 12
                                and len(pend_av) > 1):
                            emit_av()
                    op_prev[0] = op_t[0]

                # drain the final unit's pipeline; both epilogue halves
                # first so the DVE normalization chain runs back-to-back
                # (nothing else needs the op psum), then the v0 leftovers
                # keep the PE hot until the v1 output projection unblocks
                while pend_av:
                    emit_av()
                last = nunits - 1
                epilogue(last, op_prev[0], 0, 512)
                epilogue(last, op_prev[0], 512, IH)
                for _, q in outproj_v0:
                    q()

            # ---- tail: remaining output projection ----
            with nc.named_scope("outproj"):
                for _, q in outproj_v1:
                    q()

    nc.compile()
    return nc


def _get_nc():
    if "nc" not in _cache:
        _cache["nc"] = _build_nc()
    return _cache["nc"]


def _fold(a, nblk):
    """[nblk*128, C] -> [128, nblk*C] with d-major column blocks."""
    r, c = a.shape
    assert r == nblk * 128
    return np.ascontiguousarray(
        a.reshape(nblk, 128, c).transpose(1, 0, 2).reshape(128, nblk * c))


def _in_maps(x, w_qkv, w_out):
    x = np.asarray(x, dtype=np.float32)
    w_qkv = np.asarray(w_qkv, dtype=np.float32)
    w_out = np.asarray(w_out, dtype=np.float32)
    maps = []
    for c in range(NCORES):
        b, qh = c // 2, c % 2
        r0 = qh * DQ

        def fold_m(a, m):  # [512, 256] -> [128, 512] (d-major, head-pair m)
            return np.concatenate(
                [a[128 * d:128 * (d + 1), 128 * m:128 * (m + 1)]
                 for d in range(4)], axis=1)

        wk = w_qkv[D + r0:D + r0 + DQ].T                # [512, 256]
        wq = w_qkv[r0:r0 + DQ].T
        wv = _fold(w_qkv[2 * D + r0:2 * D + r0 + DQ].T, 4)
        wo = _fold(w_out[:, r0:r0 + DQ].T, 2)           # [128, 1024]
        xT = x[b].T                                     # [512, 2048]
        maps.append({
            "wkqAT": np.concatenate(
                [fold_m(wk, 0), fold_m(wq, 0)], axis=1).astype(np.float16),
            "wkqBT": np.concatenate(
                [fold_m(wk, 1), fold_m(wq, 1)], axis=1).astype(np.float16),
            "wvT": wv.astype(np.float16),
            "woT": wo.astype(np.float16),
            **{f"xQ{r}": _fold(
                xT[:, r * 512:(r + 1) * 512], 4).astype(np.float16)
               for r in range(4)},
        })
    return maps


def _gather(results):
    out = np.empty((B, S, D), np.float32)
    for b in range(B):
        acc = np.zeros((512, 2048), np.float32)
        for c in (2 * b, 2 * b + 1):
            o = results[c]["outT"].astype(np.float32)   # [128, 8192]
            acc += o.reshape(128, 4, 2048).transpose(1, 0, 2).reshape(
                512, 2048)
        out[b] = acc.T
    return out


def run(x, w_qkv, w_out, trace=False):
    from concourse.bass_utils import run_bass_kernel_spmd

    nc = _get_nc()
    res = run_bass_kernel_spmd(
        nc, _in_maps(x, w_qkv, w_out), core_ids=list(range(NCORES)), trace=trace,
    )
    return _gather(res.results), res


def kernel(x, w_qkv, w_out):
    out, _ = run(x, w_qkv, w_out)
    return out


# revision 27
# speedup vs baseline: 1.1294x; 1.0031x over previous
"""Multi-head attention (B=4, S=2048, D=512, H=8) on 8 trn2 cores.

Sharding: core c handles batch b=c//2 and the head-quad qh=c%2 (heads
4*qh..4*qh+3). Each core computes q/k/v projections for its 4 heads over the
full sequence, flash-style attention (scores kept transposed [j, i] so all
matmul contractions land on the partition dim with zero on-device transposes),
and the partial output projection over its 256 o-dims. The host pre-packs
x/weight slices into sbuf-layout 2D dram tensors (free) and sums/transposes
the two partial outputs per batch.

Design (single fused pipeline, ~192us vs 204us for the v1 schedule; the
slot period is co-saturated: ACT exp ~1.10us busy/slot, PE scores+AV
~1.07us + ~0.2us/slot of drip-fed projection work):
 - 128 slots of [128,1024] exp are the ACT floor (~140us busy); the PE
   floor is slightly higher (scores+AV at the 128-outputs/cycle roofline
   plus ~0.1us/slot of stationary-swap drain the in-order self-loading
   matmul stream cannot hide, plus ~28us of projections). Everything is
   scheduled to keep both streams dense.
 - Scores are emitted one slot AHEAD of their exp (scores(s+1) before
   AV(s-4) in the PE stream) so the exp never sits on the scores-
   completion semaphore; at bufs=12 decouples the exp WAR from AV jitter.
 - Input DMAs: host packs weights and x into contiguous [128, N] dram
   tensors matching sbuf layout exactly (8KB/4KB descriptors, 9 triggers).
   The 16 DMA engines drain all queued transfers round-robin, so the
   non-critical half of the stream is token-gated behind the critical
   half's completion (GPSIMD copies into each gated dst) - first exp fires
   ~12us in instead of ~24us. A deeper gate chain measured faster on the
   best core but blows up tail-core variance; keep one level.
 - Unit order h0v0 h1v0 h0v1 h1v1 h2v0 h3v0 h2v1 h3v1 defers every m=1
   projection past slot 64, so the deadline-sorted deferred-work queue
   stays at <=1.5 quanta/slot (v-major order peaked at 2.4/slot in units
   0-2 and gapped the exp stream there). Boundary slots jc 0-1 carry two
   AV drains each and take no quanta; jc 2-3 carry none and take up to 3.
   PSUM: sp [128,1024]x2 + op [128,1024]x1 + sc [128,512]x2 = 8 banks.
 - Softmax normalization without DRAM round-trips: each v block carries 64
   ones-columns ([128,128] stationary = 64 ones | 64 v), so the AV matmul
   replicates the softmax denominator into op psum rows 0..63 for free.
   The epilogue reads op PSUM directly: reciprocal_approx_fast (base-0
   partitions, 18-bit exact; sums are ~[1,1e20], far from its denorm/inf
   edge cases) then one multiply; the next unit's AV start=True write
   WAR-waits on those two reads. Emitted at jc==1 of the next unit, right
   after the previous unit's last AV drains.
 - Tail: both epilogue halves back-to-back, outproj psum->sbuf casts
   alternate ACT/DVE (both idle there; Copy shares the exp act table so no
   reload), and each output scq ships as two 256KB m-half DMAs so the
   first half is in flight while the second half's casts run.
 - fp16 for the score path (x, w_qkv, q, k, w_out, o): 1 cycle/row on the
   PE like bf16 but 8x the mantissa (bf16 q/k fails the 2e-2 gate at
   ~2.2e-2; fp16 lands at 3.3e-3). exp output (attn weights) stays bf16
   for fp32 exponent range since softmax skips max-subtraction, psum stays
   fp32. Output partials are written fp16 (host sums in fp32), halving the
   output DMA.
"""
import sys

sys.path.insert(0, "/opt/trn_rl_repo")
import numpy as np

B, S, D, H, HD = 4, 2048, 512, 8, 64
HPC = 4          # heads per core
DQ = HPC * HD    # 256 projection dims per core
NCORES = 8
VW = 2 * HD      # v block width: 64 v-dims + 64 ones columns (128)
IH = S // 2      # i-half processed per attention unit (1024)
AV_LAG = 4       # attn@v trails scores by this many j-chunks

_cache = {}


def _build_nc():
    import concourse.bacc as bacc
    import concourse.mybir as mybir
    import concourse.tile as tile

    F32 = mybir.dt.float32
    F16, BF16 = mybir.dt.float16, mybir.dt.bfloat16
    EXP = mybir.ActivationFunctionType.Exp

    nc = bacc.Bacc("TRN2", target_bir_lowering=False, debug=False)

    # host-packed inputs, all contiguous 2D [128, N] in final sbuf layout:
    # wkqA/B: [k d-blocks (4x128) | q d-blocks (4x128)] for head-pair m=0/1
    # wvT: [v d-blocks (4x256)]
    # woT: [wo kc-blocks (2x512)]
    # xQ0..3: x column-quarters, d-major blocks of 512 cols each
    wkqAT = nc.dram_tensor("wkqAT", [128, 1024], F16, kind="ExternalInput")
    wkqBT = nc.dram_tensor("wkqBT", [128, 1024], F16, kind="ExternalInput")
    wvT = nc.dram_tensor("wvT", [128, 1024], F16, kind="ExternalInput")
    woT = nc.dram_tensor("woT", [128, 1024], F16, kind="ExternalInput")
    xQ = [nc.dram_tensor(f"xQ{r}", [128, 2048], F16, kind="ExternalInput")
          for r in range(4)]
    # output: block m (cols m*2048+s) holds out-dims m*128..m*128+127
    outT = nc.dram_tensor("outT", [128, 8192], F16, kind="ExternalOutput")

    with tile.TileContext(nc) as tc:
        with tc.tile_pool(name="sb", bufs=1) as sb, \
             tc.tile_pool(name="ps", bufs=1, space="PSUM") as pp:
            # ---- persistent sbuf tensors ----
            wkq = [sb.tile([128, 1024], F16, tag=f"wkq{m}", name=f"wkq{m}")
                   for m in range(2)]
            wvs = sb.tile([128, 1024], F16, tag="wvs", name="wvs")
            wos = sb.tile([128, 1024], F16, tag="wos", name="wos")
            xtf = sb.tile([128, 8192], F16, tag="xtf", name="xtf")

            def wk_v(d, m):      # k weights, d-block, head-pair m
                return wkq[m][:, d * 128:(d + 1) * 128]

            def wq_v(d, m):
                return wkq[m][:, 512 + d * 128:512 + (d + 1) * 128]

            def wv_v(d, hp):     # v weights, d-block, head-pair hp
                return wvs[:, d * 256 + hp * 128:d * 256 + (hp + 1) * 128]

            def wo_v(kc, m):
                return wos[:, kc * 512 + m * 128:kc * 512 + (m + 1) * 128]

            def xts(d, c0, c1):  # x cols c0:c1 of d-block (one 512-region)
                r = c0 // 512
                assert (c1 - 1) // 512 == r
                base = r * 2048 + d * 512 + (c0 - r * 512)
                return xtf[:, base:base + (c1 - c0)]

            # ---- input DMAs ----
            # The 16 DMA engines service all queued transfers round-robin by
            # descriptor, so every queued dma_start completes only near the
            # end of the whole in-flight stream. Free-run only what the
            # prologue needs (wkqA, xQ0, wv, xQ1 = 1.5MB); gate the rest
            # behind an xQ1-completion token (tiny GPSIMD copies into each
            # gated dst chain the gated DMA's WAR dependency to xQ1). A
            # deeper 3-level chain measured FASTER on the best core but blew
            # up tail-core variance (late levels miss defq deadlines), so
            # stay at one level.
            nc.sync.dma_start(out=wkq[0][:], in_=wkqAT[:, :])
            nc.sync.dma_start(out=xtf[:, 0:2048], in_=xQ[0][:, :])
            nc.sync.dma_start(out=wvs[:], in_=wvT[:, :])
            nc.sync.dma_start(out=xtf[:, 2048:4096], in_=xQ[1][:, :])
            tok1 = xtf[0:1, 2048:2049]    # written by xQ1
            for dst in (xtf[0:1, 4096:4097], xtf[0:1, 6144:6145],
                        wkq[1][0:1, 0:1], wos[0:1, 0:1]):
                nc.gpsimd.tensor_copy(out=dst, in_=tok1)
            nc.sync.dma_start(out=xtf[:, 4096:6144], in_=xQ[2][:, :])
            nc.sync.dma_start(out=xtf[:, 6144:8192], in_=xQ[3][:, :])
            nc.sync.dma_start(out=wkq[1][:], in_=wkqBT[:, :])
            nc.sync.dma_start(out=wos[:], in_=woT[:, :])

            qT = [sb.tile([128, S], F16, tag=f"qT{m}", name=f"qT{m}")
                  for m in range(2)]
            kT = [sb.tile([128, S], F16, tag=f"kT{m}", name=f"kT{m}")
                  for m in range(2)]
            # vv block for (jc, h): cols [0:64] = ones, [64:128] = v dims
            # (ones first so the softmax sums land at psum partitions 0:64,
            # where the base-0 custom-DVE fast reciprocal can read them)
            vv = sb.tile([128, 16 * HPC * VW], BF16, tag="vv", name="vv")
            # oTn[p]: heads (2p, 2p+1) stacked on partitions; outproj moving
            oTn = [sb.tile([128, S], F16, tag=f"oTn{p}", name=f"oTn{p}")
                   for p in range(2)]

            # ---- prologue scratch: ACT table preload + PE warm-up ----
            wub = sb.tile([128, 512], BF16, tag="wub", name="wub")
            nc.vector.memset(wub[:], 0.25)
            # tiny exp: forces the ACT exp table load off the critical path
            dummy_at = sb.tile([128, 16], BF16, tag="dummy_at", name="dummy_at")
            nc.scalar.activation(dummy_at[:], wub[:, 0:16], EXP)
            ones32 = sb.tile([128, 1], F32, tag="ones32", name="ones32")
            nc.vector.memset(ones32[:], 1.0)
            vv_ones = vv[:, :].rearrange("p (g w) -> p g w", w=VW)[:, :, 0:HD]
            nc.vector.tensor_copy(
                out=vv_ones, in_=ones32[:].to_broadcast((128, 16 * HPC, HD)))
            # keep the PE busy/ramping (HAM warm) while the critical input
            # DMAs stream in; 256-wide quanta so the tail of the warm-up
            # doesn't delay the first projection once data lands
            for _ in range(16):
                wups = pp.tile([128, 512], F32, tag="sc", bufs=2, name="wups")
                nc.tensor.matmul(wups[:, 0:256], wub[:, 0:128],
                                 wub[:, 0:256],
                                 start=True, stop=True, skip_group_check=True)

            # ---- work-group emitters ----
            def qk_cast(nm, m, sc, ps):
                tgt = (qT if nm == "q" else kT)[m]
                nc.vector.tensor_copy(
                    out=tgt[:, sc * 512:(sc + 1) * 512], in_=ps[:, 0:512])
                qk_done.add((nm, m, sc))

            def qk_group(nm, m, sc):
                """whole q/k projection group: 4 matmuls + cast (prologue)."""
                ps = pp.tile([128, 512], F32, tag="sc", bufs=2, name="ps")
                wf = wq_v if nm == "q" else wk_v
                for d in range(4):
                    nc.tensor.matmul(
                        ps[:, 0:512], wf(d, m),
                        xts(d, sc * 512, (sc + 1) * 512),
                        start=(d == 0), stop=(d == 3))
                qk_cast(nm, m, sc, ps)

            def qk_quanta(nm, m, sc, deadline):
                """same group cut into 4 one-matmul quanta for the defq,
                with staggered deadlines so pops stay ~1/slot."""
                state = {}
                def q(d):
                    def emit():
                        if d == 0:
                            state["ps"] = pp.tile([128, 512], F32, tag="sc",
                                                  bufs=2, name="psq")
                        ps = state["ps"]
                        wf = wq_v if nm == "q" else wk_v
                        nc.tensor.matmul(
                            ps[:, 0:512], wf(d, m),
                            xts(d, sc * 512, (sc + 1) * 512),
                            start=(d == 0), stop=(d == 3),
                            skip_group_check=True)
                        if d == 3:
                            qk_cast(nm, m, sc, ps)
                    return emit
                return [(deadline - (3 - d), q(d)) for d in range(4)]

            def v_emit(hp, jc):
                """v projection for head-pair hp, j-chunk jc (prologue)."""
                ps = pp.tile([128, 512], F32, tag="sc", bufs=2, name="psv")
                for d in range(4):
                    nc.tensor.matmul(
                        ps[:, 0:128], xts(d, jc * 128, (jc + 1) * 128),
                        wv_v(d, hp),
                        start=(d == 0), stop=(d == 3),
                        skip_group_check=True)
                base = jc * HPC * VW + hp * 2 * VW
                out_view = vv[:, base:base + 2 * VW].rearrange(
                    "p (h w) -> p h w", w=VW)[:, :, HD:VW]
                nc.vector.tensor_copy(
                    out=out_view,
                    in_=ps[:, 0:128].rearrange("p (h d) -> p h d", d=HD))
                vv_done.add((hp, jc))

            def v_quanta(hp, jc, deadline):
                """same, split into two 2-matmul quanta for smoothing."""
                state = {}
                def q(g):
                    def emit():
                        if g == 0:
                            state["ps"] = pp.tile([128, 512], F32, tag="sc",
                                                  bufs=2, name="psv")
                        ps = state["ps"]
                        for d in (2 * g, 2 * g + 1):
                            nc.tensor.matmul(
                                ps[:, 0:128], xts(d, jc * 128, (jc + 1) * 128),
                                wv_v(d, hp),
                                start=(d == 0), stop=(d == 3),
                                skip_group_check=True)
                        if g == 1:
                            base = jc * HPC * VW + hp * 2 * VW
                            out_view = vv[:, base:base + 2 * VW].rearrange(
                                "p (h w) -> p h w", w=VW)[:, :, HD:VW]
                            nc.vector.tensor_copy(
                                out=out_view,
                                in_=ps[:, 0:128].rearrange(
                                    "p (h d) -> p h d", d=HD))
                            vv_done.add((hp, jc))
                    return emit
                return [(deadline - 1, q(0)), (deadline, q(1))]

            # per-scq gather buffers for the output projection: the 4
            # m-groups of one scq cast into one [128, 2048] tile, then ONE
            # dma_start ships all 512KB (one Sync trigger instead of 4)
            ob_state = {}
            # emission-order guards: the tile tracker links readers only to
            # PRIOR writes, so a consumer emitted before its producer is a
            # silent race. Record what has been emitted; assert before use.
            vv_done = set()
            qk_done = set()

            def out_quanta(m, scq, deadline, pool_tag="sc", cast_eng="dve"):
                """output projection group: 2 matmul quanta + cast (+dma
                after the scq's 4th group)."""
                state = {}
                def q(kc):
                    def emit():
                        if kc == 0:
                            if pool_tag == "sp":
                                state["ps"] = pp.tile([128, IH], F32,
                                                      tag="sp", bufs=2,
                                                      name="psot")
                            else:
                                state["ps"] = pp.tile([128, 512], F32,
                                                      tag="sc", bufs=2,
                                                      name="pso")
                        ps = state["ps"]
                        nc.tensor.matmul(
                            ps[:, 0:512], wo_v(kc, m),
                            oTn[kc][:, scq * 512:(scq + 1) * 512],
                            start=(kc == 0), stop=(kc == 1),
                            skip_group_check=True)
                        if kc == 1:
                            if scq not in ob_state:
                                ob_state[scq] = [sb.tile(
                                    [128, 2048], F16, tag="ob", bufs=2,
                                    name=f"ob{scq}"), 0]
                            obt, _ = ob_state[scq]
                            dst = obt[:, m * 512:(m + 1) * 512]
                            if cast_eng == "act":
                                nc.scalar.activation(
                                    dst, ps[:, 0:512],
                                    mybir.ActivationFunctionType.Copy)
                            else:
                                nc.vector.tensor_copy(
                                    out=dst, in_=ps[:, 0:512])
                            ob_state[scq][1] += 1
                            # ship in m-halves so the first 256KB is in
                            # flight while the second half's casts run
                            if ob_state[scq][1] in (2, 4):
                                mh = ob_state[scq][1] // 2 - 1  # 0 or 1
                                nc.sync.dma_start(
                                    out=outT[:, :].rearrange(
                                        "p (m s) -> p m s", s=2048)[
                                        :, 2 * mh:2 * mh + 2,
                                        scq * 512:(scq + 1) * 512],
                                    in_=obt[:, mh * 1024:(mh + 1) * 1024]
                                    .rearrange("p (m s) -> p m s", s=512))
                    return emit
                return [(deadline, q(0)), (deadline, q(1))]

            # ---- prologue projections: everything unit (0,0) needs that
            # only depends on the first two x column-quarters ----
            with nc.named_scope("proj"):
                qk_group("k", 0, 0)
                qk_group("q", 0, 0)
                qk_group("q", 0, 1)
                for jc in range(3):
                    v_emit(0, jc)

            # ---- deferred-work queue: (deadline_slot, emit) sorted ----
            defq = []
            defq += qk_quanta("k", 0, 1, 4)      # scores(0,0) jc>=4
            defq += qk_quanta("k", 0, 2, 8)      # scores(0,0) jc>=8
            defq += qk_quanta("k", 0, 3, 12)
            # AV(u, jc) normally pops at slot jc+AV_LAG, but the unit
            # boundary drains the last 4 AVs early (2/slot at jc 0-1), so
            # clamp the tail deadlines below that
            for jc in range(3, 16):
                defq += v_quanta(0, jc, jc + AV_LAG if jc < 12 else jc - 1)
            # unit order defers all m=1 tensors to slot 64+, so the deferred
            # work spreads at <=1.5 pops/slot instead of 2.4 in units 0-2:
            #   units: h0v0 h1v0 | h0v1 h1v1 | h2v0 h3v0 | h2v1 h3v1
            defq += qk_quanta("q", 0, 2, 27)     # unit 2 scores @ slot 31
            defq += qk_quanta("q", 0, 3, 31)
            for jc in range(16):                 # vv hp1 for unit 4 @ 64+
                defq += v_quanta(1, jc, 34 + (27 * jc) // 16)
            defq += qk_quanta("q", 1, 0, 56)     # unit 4 scores @ slot 63
            defq += qk_quanta("q", 1, 1, 60)
            defq += qk_quanta("k", 1, 0, 61)
            defq += qk_quanta("k", 1, 1, 64)
            defq += qk_quanta("k", 1, 2, 68)
            defq += qk_quanta("k", 1, 3, 72)
            defq += qk_quanta("q", 1, 2, 88)     # unit 6 scores @ slot 95
            defq += qk_quanta("q", 1, 3, 92)
            defq.sort(key=lambda t: t[0])
            outproj_v0 = []   # gated on epilogue of unit 5 (h3,v0) ~slot 97
            for scq in range(2):      # scq-major so the combined per-scq
                for m in range(4):    # DMA fires right after its 4th group
                    outproj_v0 += out_quanta(m, scq, 120)
            outproj_v1 = []   # tail: needs the last unit's epilogue.
            # scq-major: the first half only reads columns the first
            # epilogue-half has normalized; casts alternate ACT/DVE (both
            # idle in the tail) so the cast chain halves
            gi = 0
            for scq in range(2, 4):
                for m in range(4):
                    outproj_v1 += out_quanta(
                        m, scq, 999, pool_tag=("sp" if gi % 2 else "sc"),
                        cast_eng=("act" if m % 2 == 0 else "dve"))
                    gi += 1

            # ---- attention: unit order keeps head-pair 0 first (defers the
            # m=1 projections), v0 before v1 within each pair so outproj
            # halves unlock as early as possible ----
            units = [(0, 0), (1, 0), (0, 1), (1, 1),
                     (2, 0), (3, 0), (2, 1), (3, 1)]
            nunits = len(units)

            def epilogue(uid, op, c0=0, c1=IH):
                """normalize straight out of op psum: rows 0..63 hold the
                softmax denominator replicated via the ones-columns of vv,
                rows 64..127 the unnormalized output. The next unit's AV
                start=True write WAR-waits on these two reads."""
                h, v = units[uid]
                p, off = h // 2, 64 * (h % 2)
                recip = sb.tile([64, IH], F32, tag="recip", bufs=2,
                                name="recip")
                nc.vector.reciprocal_approx_fast(
                    out=recip[:, c0:c1], in_=op[0:64, c0:c1])
                nc.vector.tensor_mul(
                    out=oTn[p][off:off + 64, v * IH + c0:v * IH + c1],
                    in0=op[64:128, c0:c1], in1=recip[:, c0:c1])

            with nc.named_scope("attn"):
                pend_av = []       # (uid, h, jc, at_tile, op_tile-or-None)
                op_t = [None]      # current unit's op accumulator
                op_prev = [None]
                sp_pend = {}       # s -> sp tile (scores emitted, exp not)

                def emit_scores(s):
                    """scores [j(128), i(1024)] for global slot s. Emitted
                    one slot AHEAD of its exp so the exp never waits on the
                    scores-completion semaphore (~150ns/slot otherwise)."""
                    uu, jj = s // 16, s % 16
                    hh, vv_ = units[uu]
                    mm, oo, ii0 = hh // 2, 64 * (hh % 2), vv_ * IH
                    assert ("k", mm, jj // 4) in qk_done, ("k", s, mm, jj)
                    sp = pp.tile([128, IH], F32, tag="sp", bufs=2,
                                 name="sp")
                    for scc in range(2):
                        assert ("q", mm, (ii0 + scc * 512) // 512) in qk_done, \
                            ("q", s, mm, ii0, scc)
                        nc.tensor.matmul(
                            sp[:, scc * 512:(scc + 1) * 512],
                            kT[mm][oo:oo + 64, jj * 128:(jj + 1) * 128],
                            qT[mm][oo:oo + 64,
                                   ii0 + scc * 512:ii0 + (scc + 1) * 512],
                            start=True, stop=True)
                    sp_pend[s] = sp

                def emit_av():
                    _, hh, jj, aa, oo = pend_av.pop(0)
                    if oo is None:
                        oo = op_t[0]
                    assert (hh // 2, jj) in vv_done, ("vv", hh, jj)
                    base = jj * HPC * VW + hh * VW
                    for scc in range(2):
                        nc.tensor.matmul(
                            oo[:, scc * 512:(scc + 1) * 512],
                            vv[:, base:base + VW],
                            aa[:, scc * 512:(scc + 1) * 512],
                            start=(jj == 0), stop=(jj == 15),
                            skip_group_check=True)

                emit_scores(0)
                for ui, (h, v) in enumerate(units):
                    for jc in range(16):
                        s = ui * 16 + jc
                        at_t = sb.tile([128, IH], BF16, tag="at", bufs=12,
                                       name="at")
                        nc.scalar.activation(at_t[:], sp_pend.pop(s), EXP)
                        if s + 1 < 16 * nunits:
                            emit_scores(s + 1)
                        # acquire op right before this unit's first AV (and
                        # after the previous unit's epilogue was emitted)
                        if jc == AV_LAG:
                            op_t[0] = pp.tile([128, IH], F32, tag="op",
                                              bufs=1, name="op")
                        pend_av.append(
                            (ui, h, jc, at_t,
                             op_t[0] if jc >= AV_LAG else None))
                        # deferred proj/outproj quanta in the PE slack.
                        # Boundary slots jc 0-1 carry 2 AV drains each (PE
                        # ~1.6us, over budget) -> no pops there; jc 2-3
                        # carry no AV at all (~540ns slack) -> up to 3 pops.
                        # Net over the 4 boundary slots the PE load is flat.
                        popped = 0
                        cap = 1
                        if ui > 0 and jc < 2:
                            cap = 0
                        elif ui > 0 and jc in (2, 3):
                            cap = 3
                        while defq and (popped < cap
                                        or defq[0][0] <= s + (2 if cap else 0)):
                            defq.pop(0)[1]()
                            popped += 1
                        if not defq and outproj_v0 and s >= 99:
                            want = min(cap - popped, len(outproj_v0))
                            for _ in range(max(0, want)):
                                outproj_v0.pop(0)[1]()
                        if len(pend_av) > AV_LAG:
                            emit_av()
                        # boundary: drain the previous unit's AVs 2/slot,
                        # then emit its epilogue right after its last AV so
                        # the op psum is free before this unit's first AV
                        # (popped at jc==AV_LAG) needs the banks
                        if (ui > 0 and jc < 2 and pend_av
                                and pend_av[0][0] == ui - 1):
                            emit_av()
                            if jc == 1:
                                while pend_av and pend_av[0][0] == ui - 1:
                                    emit_av()
                                epilogue(ui - 1, op_prev[0])
                        # last unit: pre-drain so the tail chain is short
                        if (ui == nunits - 1 and jc >= 12
                                and len(pend_av) > 1):
                            emit_av()
                    op_prev[0] = op_t[0]

                # drain the final unit's pipeline; both epilogue halves
                # first so the DVE normalization chain runs back-to-back
                # (nothing else needs the op psum), then the v0 leftovers
                # keep the PE hot until the v1 output projection unblocks
                while pend_av:
                    emit_av()
                last = nunits - 1
                epilogue(last, op_prev[0], 0, 512)
                epilogue(last, op_prev[0], 512, IH)
                for _, q in outproj_v0:
                    q()

            # ---- tail: remaining output projection ----
            with nc.named_scope("outproj"):
                for _, q in outproj_v1:
                    q()

    nc.compile()
    return nc


def _get_nc():
    if "nc" not in _cache:
        _cache["nc"] = _build_nc()
    return _cache["nc"]


def _fold(a, nblk):
    """[nblk*128, C] -> [128, nblk*C] with d-major column blocks."""
    r, c = a.shape
    assert r == nblk * 128
    return np.ascontiguousarray(
        a.reshape(nblk, 128, c).transpose(1, 0, 2).reshape(128, nblk * c))


def _in_maps(x, w_qkv, w_out):
    x = np.asarray(x, dtype=np.float32)
    w_qkv = np.asarray(w_qkv, dtype=np.float32)
    w_out = np.asarray(w_out, dtype=np.float32)
    maps = []
    for c in range(NCORES):
        b, qh = c // 2, c % 2
        r0 = qh * DQ

        def fold_m(a, m):  # [512, 256] -> [128, 512] (d-major, head-pair m)
            return np.concatenate(
                [a[128 * d:128 * (d + 1), 128 * m:128 * (m + 1)]
                 for d in range(4)], axis=1)

        wk = w_qkv[D + r0:D + r0 + DQ].T                # [512, 256]
        wq = w_qkv[r0:r0 + DQ].T
        wv = _fold(w_qkv[2 * D + r0:2 * D + r0 + DQ].T, 4)
        wo = _fold(w_out[:, r0:r0 + DQ].T, 2)           # [128, 1024]
        xT = x[b].T                                     # [512, 2048]
        maps.append({
            "wkqAT": np.concatenate(
                [fold_m(wk, 0), fold_m(wq, 0)], axis=1).astype(np.float16),
            "wkqBT": np.concatenate(
                [fold_m(wk, 1), fold_m(wq, 1)], axis=1).astype(np.float16),
            "wvT": wv.astype(np.float16),
            "woT": wo.astype(np.float16),
            **{f"xQ{r}": _fold(
                xT[:, r * 512:(r + 1) * 512], 4).astype(np.float16)
               for r in range(4)},
        })
    return maps


def _gather(results):
    out = np.empty((B, S, D), np.float32)
    for b in range(B):
        acc = np.zeros((512, 2048), np.float32)
        for c in (2 * b, 2 * b + 1):
            o = results[c]["outT"].astype(np.float32)   # [128, 8192]
            acc += o.reshape(128, 4, 2048).transpose(1, 0, 2).reshape(
                512, 2048)
        out[b] = acc.T
    return out


def run(x, w_qkv, w_out, trace=False):
    from concourse.bass_utils import run_bass_kernel_spmd

    nc = _get_nc()
    res = run_bass_kernel_spmd(
        nc, _in_maps(x, w_qkv, w_out), core_ids=list(range(NCORES)), trace=trace,
    )
    return _gather(res.results), res


def kernel(x, w_qkv, w_out):
    out, _ = run(x, w_qkv, w_out)
    return out


# revision 29
# speedup vs baseline: 1.1332x; 1.0034x over previous
"""Multi-head attention (B=4, S=2048, D=512, H=8) on 8 trn2 cores.

Sharding: core c handles batch b=c//2 and the head-quad qh=c%2 (heads
4*qh..4*qh+3). Each core computes q/k/v projections for its 4 heads over the
full sequence, flash-style attention (scores kept transposed [j, i] so all
matmul contractions land on the partition dim with zero on-device transposes),
and the partial output projection over its 256 o-dims. The host pre-packs
x/weight slices into sbuf-layout 2D dram tensors (free) and sums/transposes
the two partial outputs per batch.

Design (single fused pipeline, ~192us vs 204us for the v1 schedule; the
slot period is co-saturated: ACT exp ~1.10us busy/slot, PE scores+AV
~1.07us + ~0.2us/slot of drip-fed projection work):
 - 128 slots of [128,1024] exp are the ACT floor (~140us busy); the PE
   floor is slightly higher (scores+AV at the 128-outputs/cycle roofline
   plus ~0.1us/slot of stationary-swap drain the in-order self-loading
   matmul stream cannot hide, plus ~28us of projections). Everything is
   scheduled to keep both streams dense.
 - Scores are emitted one slot AHEAD of their exp (scores(s+1) before
   AV(s-4) in the PE stream) so the exp never sits on the scores-
   completion semaphore; at bufs=12 decouples the exp WAR from AV jitter.
 - Input DMAs: host packs weights and x into contiguous [128, N] dram
   tensors matching sbuf layout exactly (8KB/4KB descriptors, 9 triggers).
   The 16 DMA engines drain all queued transfers round-robin, so the
   non-critical half of the stream is token-gated behind the critical
   half's completion (GPSIMD copies into each gated dst) - first exp fires
   ~12us in instead of ~24us. A deeper gate chain measured faster on the
   best core but blows up tail-core variance; keep one level.
 - Unit order h0v0 h1v0 h0v1 h1v1 h2v0 h3v0 h2v1 h3v1 defers every m=1
   projection past slot 64, so the deadline-sorted deferred-work queue
   stays at <=1.5 quanta/slot (v-major order peaked at 2.4/slot in units
   0-2 and gapped the exp stream there). Boundary slots jc 0-1 carry two
   AV drains each and take no quanta; jc 2-3 carry none and take up to 3.
   PSUM: sp [128,1024]x2 + op [128,1024]x1 + sc [128,512]x2 = 8 banks.
 - Softmax normalization without DRAM round-trips: each v block carries 64
   ones-columns ([128,128] stationary = 64 ones | 64 v), so the AV matmul
   replicates the softmax denominator into op psum rows 0..63 for free.
   The epilogue reads op PSUM directly: reciprocal_approx_fast (base-0
   partitions, 18-bit exact; sums are ~[1,1e20], far from its denorm/inf
   edge cases) then one multiply; the next unit's AV start=True write
   WAR-waits on those two reads. Emitted at jc==1 of the next unit, right
   after the previous unit's last AV drains.
 - Tail: both epilogue halves back-to-back, outproj psum->sbuf casts
   alternate ACT/DVE (both idle there; Copy shares the exp act table so no
   reload), and each output scq ships as two 256KB m-half DMAs so the
   first half is in flight while the second half's casts run.
 - fp16 for the score path (x, w_qkv, q, k, w_out, o): 1 cycle/row on the
   PE like bf16 but 8x the mantissa (bf16 q/k fails the 2e-2 gate at
   ~2.2e-2; fp16 lands at 3.3e-3). exp output (attn weights) stays bf16
   for fp32 exponent range since softmax skips max-subtraction, psum stays
   fp32. Output partials are written fp16 (host sums in fp32), halving the
   output DMA.
"""
import sys

sys.path.insert(0, "/opt/trn_rl_repo")
import numpy as np

B, S, D, H, HD = 4, 2048, 512, 8, 64
HPC = 4          # heads per core
DQ = HPC * HD    # 256 projection dims per core
NCORES = 8
VW = 2 * HD      # v block width: 64 v-dims + 64 ones columns (128)
IH = S // 2      # i-half processed per attention unit (1024)
AV_LAG = 4       # attn@v trails scores by this many j-chunks

_cache = {}


def _build_nc():
    import concourse.bacc as bacc
    import concourse.mybir as mybir
    import concourse.tile as tile

    F32 = mybir.dt.float32
    F16, BF16 = mybir.dt.float16, mybir.dt.bfloat16
    EXP = mybir.ActivationFunctionType.Exp

    nc = bacc.Bacc("TRN2", target_bir_lowering=False, debug=False)

    # host-packed inputs, all contiguous 2D [128, N] in final sbuf layout:
    # wkqA/B: [k d-blocks (4x128) | q d-blocks (4x128)] for head-pair m=0/1
    # wvT: [v d-blocks (4x256)]
    # woT: [wo kc-blocks (2x512)]
    # xQ0..3: x column-quarters, d-major blocks of 512 cols each
    wkqAT = nc.dram_tensor("wkqAT", [128, 1024], F16, kind="ExternalInput")
    wkqBT = nc.dram_tensor("wkqBT", [128, 1024], F16, kind="ExternalInput")
    wvT = nc.dram_tensor("wvT", [128, 1024], F16, kind="ExternalInput")
    woT = nc.dram_tensor("woT", [128, 1024], F16, kind="ExternalInput")
    xQ = [nc.dram_tensor(f"xQ{r}", [128, 2048], F16, kind="ExternalInput")
          for r in range(4)]
    # output: block m (cols m*2048+s) holds out-dims m*128..m*128+127
    outT = nc.dram_tensor("outT", [128, 8192], F16, kind="ExternalOutput")

    with tile.TileContext(nc) as tc:
        with tc.tile_pool(name="sb", bufs=1) as sb, \
             tc.tile_pool(name="ps", bufs=1, space="PSUM") as pp:
            # ---- persistent sbuf tensors ----
            wkq = [sb.tile([128, 1024], F16, tag=f"wkq{m}", name=f"wkq{m}")
                   for m in range(2)]
            wvs = sb.tile([128, 1024], F16, tag="wvs", name="wvs")
            wos = sb.tile([128, 1024], F16, tag="wos", name="wos")
            xtf = sb.tile([128, 8192], F16, tag="xtf", name="xtf")

            def wk_v(d, m):      # k weights, d-block, head-pair m
                return wkq[m][:, d * 128:(d + 1) * 128]

            def wq_v(d, m):
                return wkq[m][:, 512 + d * 128:512 + (d + 1) * 128]

            def wv_v(d, hp):     # v weights, d-block, head-pair hp
                return wvs[:, d * 256 + hp * 128:d * 256 + (hp + 1) * 128]

            def wo_v(kc, m):
                return wos[:, kc * 512 + m * 128:kc * 512 + (m + 1) * 128]

            def xts(d, c0, c1):  # x cols c0:c1 of d-block (one 512-region)
                r = c0 // 512
                assert (c1 - 1) // 512 == r
                base = r * 2048 + d * 512 + (c0 - r * 512)
                return xtf[:, base:base + (c1 - c0)]

            # ---- input DMAs ----
            # The 16 DMA engines service all queued transfers round-robin by
            # descriptor, so every queued dma_start completes only near the
            # end of the whole in-flight stream. Free-run only what the
            # prologue needs (wkqA, xQ0, wv, xQ1 = 1.5MB); gate the rest
            # behind an xQ1-completion token (tiny GPSIMD copies into each
            # gated dst chain the gated DMA's WAR dependency to xQ1). A
            # deeper 3-level chain measured FASTER on the best core but blew
            # up tail-core variance (late levels miss defq deadlines), so
            # stay at one level.
            nc.sync.dma_start(out=wkq[0][:], in_=wkqAT[:, :])
            nc.sync.dma_start(out=xtf[:, 0:2048], in_=xQ[0][:, :])
            nc.sync.dma_start(out=wvs[:], in_=wvT[:, :])
            nc.sync.dma_start(out=xtf[:, 2048:4096], in_=xQ[1][:, :])
            tok1 = xtf[0:1, 2048:2049]    # written by xQ1
            for dst in (xtf[0:1, 4096:4097], xtf[0:1, 6144:6145],
                        wkq[1][0:1, 0:1], wos[0:1, 0:1]):
                nc.gpsimd.tensor_copy(out=dst, in_=tok1)
            nc.sync.dma_start(out=xtf[:, 4096:6144], in_=xQ[2][:, :])
            nc.sync.dma_start(out=xtf[:, 6144:8192], in_=xQ[3][:, :])
            nc.sync.dma_start(out=wkq[1][:], in_=wkqBT[:, :])
            nc.sync.dma_start(out=wos[:], in_=woT[:, :])

            qT = [sb.tile([128, S], F16, tag=f"qT{m}", name=f"qT{m}")
                  for m in range(2)]
            kT = [sb.tile([128, S], F16, tag=f"kT{m}", name=f"kT{m}")
                  for m in range(2)]
            # vv block for (jc, h): cols [0:64] = ones, [64:128] = v dims
            # (ones first so the softmax sums land at psum partitions 0:64,
            # where the base-0 custom-DVE fast reciprocal can read them)
            vv = sb.tile([128, 16 * HPC * VW], BF16, tag="vv", name="vv")
            # oTn[p]: heads (2p, 2p+1) stacked on partitions; outproj moving
            oTn = [sb.tile([128, S], F16, tag=f"oTn{p}", name=f"oTn{p}")
                   for p in range(2)]

            # ---- prologue scratch: ACT table preload + PE warm-up ----
            wub = sb.tile([128, 512], BF16, tag="wub", name="wub")
            nc.vector.memset(wub[:], 0.25)
            # tiny exp: forces the ACT exp table load off the critical path
            dummy_at = sb.tile([128, 16], BF16, tag="dummy_at", name="dummy_at")
            nc.scalar.activation(dummy_at[:], wub[:, 0:16], EXP)
            ones32 = sb.tile([128, 1], F32, tag="ones32", name="ones32")
            nc.vector.memset(ones32[:], 1.0)
            vv_ones = vv[:, :].rearrange("p (g w) -> p g w", w=VW)[:, :, 0:HD]
            nc.vector.tensor_copy(
                out=vv_ones, in_=ones32[:].to_broadcast((128, 16 * HPC, HD)))
            # keep the PE busy/ramping (HAM warm) while the critical input
            # DMAs stream in; 256-wide quanta so the tail of the warm-up
            # doesn't delay the first projection once data lands
            for _ in range(20):
                wups = pp.tile([128, 512], F32, tag="sc", bufs=2, name="wups")
                nc.tensor.matmul(wups[:, 0:256], wub[:, 0:128],
                                 wub[:, 0:256],
                                 start=True, stop=True, skip_group_check=True)

            # ---- work-group emitters ----
            def qk_cast(nm, m, sc, ps, cast_eng="dve"):
                tgt = (qT if nm == "q" else kT)[m]
                if cast_eng == "act":
                    # ACT is idle in the prologue; Copy shares the exp act
                    # table so there is no table reload
                    nc.scalar.activation(
                        tgt[:, sc * 512:(sc + 1) * 512], ps[:, 0:512],
                        mybir.ActivationFunctionType.Copy)
                else:
                    nc.vector.tensor_copy(
                        out=tgt[:, sc * 512:(sc + 1) * 512], in_=ps[:, 0:512])
                qk_done.add((nm, m, sc))

            def qk_group(nm, m, sc, cast_eng="dve"):
                """whole q/k projection group: 4 matmuls + cast (prologue)."""
                ps = pp.tile([128, 512], F32, tag="sc", bufs=2, name="ps")
                wf = wq_v if nm == "q" else wk_v
                for d in range(4):
                    nc.tensor.matmul(
                        ps[:, 0:512], wf(d, m),
                        xts(d, sc * 512, (sc + 1) * 512),
                        start=(d == 0), stop=(d == 3))
                qk_cast(nm, m, sc, ps, cast_eng)

            def qk_quanta(nm, m, sc, deadline):
                """same group cut into 4 one-matmul quanta for the defq,
                with staggered deadlines so pops stay ~1/slot."""
                state = {}
                def q(d):
                    def emit():
                        if d == 0:
                            state["ps"] = pp.tile([128, 512], F32, tag="sc",
                                                  bufs=2, name="psq")
                        ps = state["ps"]
                        wf = wq_v if nm == "q" else wk_v
                        nc.tensor.matmul(
                            ps[:, 0:512], wf(d, m),
                            xts(d, sc * 512, (sc + 1) * 512),
                            start=(d == 0), stop=(d == 3),
                            skip_group_check=True)
                        if d == 3:
                            qk_cast(nm, m, sc, ps)
                    return emit
                return [(deadline - (3 - d), q(d)) for d in range(4)]

            def v_emit(hp, jc):
                """v projection for head-pair hp, j-chunk jc (prologue)."""
                ps = pp.tile([128, 512], F32, tag="sc", bufs=2, name="psv")
                for d in range(4):
                    nc.tensor.matmul(
                        ps[:, 0:128], xts(d, jc * 128, (jc + 1) * 128),
                        wv_v(d, hp),
                        start=(d == 0), stop=(d == 3),
                        skip_group_check=True)
                base = jc * HPC * VW + hp * 2 * VW
                out_view = vv[:, base:base + 2 * VW].rearrange(
                    "p (h w) -> p h w", w=VW)[:, :, HD:VW]
                nc.vector.tensor_copy(
                    out=out_view,
                    in_=ps[:, 0:128].rearrange("p (h d) -> p h d", d=HD))
                vv_done.add((hp, jc))

            def v_quanta(hp, jc, deadline):
                """same, split into two 2-matmul quanta for smoothing."""
                state = {}
                def q(g):
                    def emit():
                        if g == 0:
                            state["ps"] = pp.tile([128, 512], F32, tag="sc",
                                                  bufs=2, name="psv")
                        ps = state["ps"]
                        for d in (2 * g, 2 * g + 1):
                            nc.tensor.matmul(
                                ps[:, 0:128], xts(d, jc * 128, (jc + 1) * 128),
                                wv_v(d, hp),
                                start=(d == 0), stop=(d == 3),
                                skip_group_check=True)
                        if g == 1:
                            base = jc * HPC * VW + hp * 2 * VW
                            out_view = vv[:, base:base + 2 * VW].rearrange(
                                "p (h w) -> p h w", w=VW)[:, :, HD:VW]
                            nc.vector.tensor_copy(
                                out=out_view,
                                in_=ps[:, 0:128].rearrange(
                                    "p (h d) -> p h d", d=HD))
                            vv_done.add((hp, jc))
                    return emit
                return [(deadline - 1, q(0)), (deadline, q(1))]

            # per-scq gather buffers for the output projection: the 4
            # m-groups of one scq cast into one [128, 2048] tile, then ONE
            # dma_start ships all 512KB (one Sync trigger instead of 4)
            ob_state = {}
            # emission-order guards: the tile tracker links readers only to
            # PRIOR writes, so a consumer emitted before its producer is a
            # silent race. Record what has been emitted; assert before use.
            vv_done = set()
            qk_done = set()

            def out_quanta(m, scq, deadline, pool_tag="sc", cast_eng="dve"):
                """output projection group: 2 matmul quanta + cast (+dma
                after the scq's 4th group)."""
                state = {}
                def q(kc):
                    def emit():
                        if kc == 0:
                            if pool_tag == "sp":
                                state["ps"] = pp.tile([128, IH], F32,
                                                      tag="sp", bufs=2,
                                                      name="psot")
                            else:
                                state["ps"] = pp.tile([128, 512], F32,
                                                      tag="sc", bufs=2,
                                                      name="pso")
                        ps = state["ps"]
                        nc.tensor.matmul(
                            ps[:, 0:512], wo_v(kc, m),
                            oTn[kc][:, scq * 512:(scq + 1) * 512],
                            start=(kc == 0), stop=(kc == 1),
                            skip_group_check=True)
                        if kc == 1:
                            if scq not in ob_state:
                                ob_state[scq] = [sb.tile(
                                    [128, 2048], F16, tag="ob", bufs=2,
                                    name=f"ob{scq}"), 0]
                            obt, _ = ob_state[scq]
                            dst = obt[:, m * 512:(m + 1) * 512]
                            if cast_eng == "act":
                                nc.scalar.activation(
                                    dst, ps[:, 0:512],
                                    mybir.ActivationFunctionType.Copy)
                            else:
                                nc.vector.tensor_copy(
                                    out=dst, in_=ps[:, 0:512])
                            ob_state[scq][1] += 1
                            # ship in m-halves so the first 256KB is in
                            # flight while the second half's casts run
                            if ob_state[scq][1] in (2, 4):
                                mh = ob_state[scq][1] // 2 - 1  # 0 or 1
                                nc.sync.dma_start(
                                    out=outT[:, :].rearrange(
                                        "p (m s) -> p m s", s=2048)[
                                        :, 2 * mh:2 * mh + 2,
                                        scq * 512:(scq + 1) * 512],
                                    in_=obt[:, mh * 1024:(mh + 1) * 1024]
                                    .rearrange("p (m s) -> p m s", s=512))
                    return emit
                return [(deadline, q(0)), (deadline, q(1))]

            # ---- prologue projections: everything unit (0,0) needs that
            # only depends on the first two x column-quarters ----
            with nc.named_scope("proj"):
                qk_group("k", 0, 0, cast_eng="act")
                qk_group("q", 0, 0)
                qk_group("q", 0, 1)

            # ---- deferred-work queue: (deadline_slot, emit) sorted ----
            # v(0, 0..2) used to be prologue emissions; as deadline-0..2
            # quanta they come off the exp(0) critical path (wv lands with
            # the free DMA set, so the pops never block the PE queue)
            defq = []
            for jc in range(3):
                defq += v_quanta(0, jc, jc)
            defq += qk_quanta("k", 0, 1, 4)      # scores(0,0) jc>=4
            defq += qk_quanta("k", 0, 2, 8)      # scores(0,0) jc>=8
            defq += qk_quanta("k", 0, 3, 12)
            # AV(u, jc) normally pops at slot jc+AV_LAG, but the unit
            # boundary drains the last 4 AVs early (2/slot at jc 0-1), so
            # clamp the tail deadlines below that
            for jc in range(3, 16):
                defq += v_quanta(0, jc, jc + AV_LAG if jc < 12 else jc - 1)
            # unit order defers all m=1 tensors to slot 64+, so the deferred
            # work spreads at <=1.5 pops/slot instead of 2.4 in units 0-2:
            #   units: h0v0 h1v0 | h0v1 h1v1 | h2v0 h3v0 | h2v1 h3v1
            defq += qk_quanta("q", 0, 2, 27)     # unit 2 scores @ slot 31
            defq += qk_quanta("q", 0, 3, 31)
            for jc in range(16):                 # vv hp1 for unit 4 @ 64+
                defq += v_quanta(1, jc, 34 + (27 * jc) // 16)
            defq += qk_quanta("q", 1, 0, 56)     # unit 4 scores @ slot 63
            defq += qk_quanta("q", 1, 1, 60)
            defq += qk_quanta("k", 1, 0, 61)
            defq += qk_quanta("k", 1, 1, 64)
            defq += qk_quanta("k", 1, 2, 68)
            defq += qk_quanta("k", 1, 3, 72)
            defq += qk_quanta("q", 1, 2, 88)     # unit 6 scores @ slot 95
            defq += qk_quanta("q", 1, 3, 92)
            defq.sort(key=lambda t: t[0])
            outproj_v0 = []   # gated on epilogue of unit 5 (h3,v0) ~slot 97
            for scq in range(2):      # scq-major so the combined per-scq
                for m in range(4):    # DMA fires right after its 4th group
                    outproj_v0 += out_quanta(m, scq, 120)
            outproj_v1 = []   # tail: needs the last unit's epilogue.
            # scq-major: the first half only reads columns the first
            # epilogue-half has normalized; casts alternate ACT/DVE (both
            # idle in the tail) so the cast chain halves
            gi = 0
            for scq in range(2, 4):
                for m in range(4):
                    outproj_v1 += out_quanta(
                        m, scq, 999, pool_tag=("sp" if gi % 2 else "sc"),
                        cast_eng=("act" if m % 2 == 0 else "dve"))
                    gi += 1

            # ---- attention: unit order keeps head-pair 0 first (defers the
            # m=1 projections), v0 before v1 within each pair so outproj
            # halves unlock as early as possible ----
            units = [(0, 0), (1, 0), (0, 1), (1, 1),
                     (2, 0), (3, 0), (2, 1), (3, 1)]
            nunits = len(units)

            def epilogue(uid, op, c0=0, c1=IH):
                """normalize straight out of op psum: rows 0..63 hold the
                softmax denominator replicated via the ones-columns of vv,
                rows 64..127 the unnormalized output. The next unit's AV
                start=True write WAR-waits on these two reads."""
                h, v = units[uid]
                p, off = h // 2, 64 * (h % 2)
                recip = sb.tile([64, IH], F32, tag="recip", bufs=2,
                                name="recip")
                nc.vector.reciprocal_approx_fast(
                    out=recip[:, c0:c1], in_=op[0:64, c0:c1])
                nc.vector.tensor_mul(
                    out=oTn[p][off:off + 64, v * IH + c0:v * IH + c1],
                    in0=op[64:128, c0:c1], in1=recip[:, c0:c1])

            with nc.named_scope("attn"):
                pend_av = []       # (uid, h, jc, at_tile, op_tile-or-None)
                op_t = [None]      # current unit's op accumulator
                op_prev = [None]
                sp_pend = {}       # s -> sp tile (scores emitted, exp not)

                def emit_scores(s):
                    """scores [j(128), i(1024)] for global slot s. Emitted
                    one slot AHEAD of its exp so the exp never waits on the
                    scores-completion semaphore (~150ns/slot otherwise)."""
                    uu, jj = s // 16, s % 16
                    hh, vv_ = units[uu]
                    mm, oo, ii0 = hh // 2, 64 * (hh % 2), vv_ * IH
                    assert ("k", mm, jj // 4) in qk_done, ("k", s, mm, jj)
                    sp = pp.tile([128, IH], F32, tag="sp", bufs=2,
                                 name="sp")
                    for scc in range(2):
                        assert ("q", mm, (ii0 + scc * 512) // 512) in qk_done, \
                            ("q", s, mm, ii0, scc)
                        nc.tensor.matmul(
                            sp[:, scc * 512:(scc + 1) * 512],
                            kT[mm][oo:oo + 64, jj * 128:(jj + 1) * 128],
                            qT[mm][oo:oo + 64,
                                   ii0 + scc * 512:ii0 + (scc + 1) * 512],
                            start=True, stop=True)
                    sp_pend[s] = sp

                def emit_av():
                    _, hh, jj, aa, oo = pend_av.pop(0)
                    if oo is None:
                        oo = op_t[0]
                    assert (hh // 2, jj) in vv_done, ("vv", hh, jj)
                    base = jj * HPC * VW + hh * VW
                    for scc in range(2):
                        nc.tensor.matmul(
                            oo[:, scc * 512:(scc + 1) * 512],
                            vv[:, base:base + VW],
                            aa[:, scc * 512:(scc + 1) * 512],
                            start=(jj == 0), stop=(jj == 15),
                            skip_group_check=True)

                emit_scores(0)
                for ui, (h, v) in enumerate(units):
                    for jc in range(16):
                        s = ui * 16 + jc
                        at_t = sb.tile([128, IH], BF16, tag="at", bufs=12,
                                       name="at")
                        nc.scalar.activation(at_t[:], sp_pend.pop(s), EXP)
                        if s + 1 < 16 * nunits:
                            emit_scores(s + 1)
                        # acquire op right before this unit's first AV (and
                        # after the previous unit's epilogue was emitted)
                        if jc == AV_LAG:
                            op_t[0] = pp.tile([128, IH], F32, tag="op",
                                              bufs=1, name="op")
                        pend_av.append(
                            (ui, h, jc, at_t,
                             op_t[0] if jc >= AV_LAG else None))
                        # deferred proj/outproj quanta in the PE slack.
                        # Boundary slots jc 0-1 carry 2 AV drains each (PE
                        # ~1.6us, over budget) -> no pops there; jc 2-3
                        # carry no AV at all (~540ns slack) -> up to 3 pops.
                        # Net over the 4 boundary slots the PE load is flat.
                        popped = 0
                        cap = 1
                        if ui > 0 and jc < 2:
                            cap = 0
                        elif ui > 0 and jc in (2, 3):
                            cap = 3
                        while defq and (popped < cap
                                        or defq[0][0] <= s + (2 if cap else 0)):
                            defq.pop(0)[1]()
                            popped += 1
                        if not defq and outproj_v0 and s >= 99:
                            want = min(cap - popped, len(outproj_v0))
                            for _ in range(max(0, want)):
                                outproj_v0.pop(0)[1]()
                        if len(pend_av) > AV_LAG:
                            emit_av()
                        # boundary: drain the previous unit's AVs 2/slot,
                        # then emit its epilogue right after its last AV so
                        # the op psum is free before this unit's first AV
                        # (popped at jc==AV_LAG) needs the banks
                        if (ui > 0 and jc < 2 and pend_av
                                and pend_av[0][0] == ui - 1):
                            emit_av()
                            if jc == 1:
                                while pend_av and pend_av[0][0] == ui - 1:
                                    emit_av()
                                epilogue(ui - 1, op_prev[0])
                        # last unit: pre-drain so the tail chain is short
                        if (ui == nunits - 1 and jc >= 12
                                and len(pend_av) > 1):
                            emit_av()
                    op_prev[0] = op_t[0]

                # drain the final unit's pipeline; both epilogue halves
                # first so the DVE normalization chain runs back-to-back
                # (nothing else needs the op psum), then the v0 leftovers
                # keep the PE hot until the v1 output projection unblocks
                while pend_av:
                    emit_av()
                last = nunits - 1
                epilogue(last, op_prev[0], 0, 512)
                epilogue(last, op_prev[0], 512, IH)
                for _, q in outproj_v0:
                    q()

            # ---- tail: remaining output projection ----
            with nc.named_scope("outproj"):
                for _, q in outproj_v1:
                    q()

    nc.compile()
    return nc


def _get_nc():
    if "nc" not in _cache:
        _cache["nc"] = _build_nc()
    return _cache["nc"]


def _fold(a, nblk):
    """[nblk*128, C] -> [128, nblk*C] with d-major column blocks."""
    r, c = a.shape
    assert r == nblk * 128
    return np.ascontiguousarray(
        a.reshape(nblk, 128, c).transpose(1, 0, 2).reshape(128, nblk * c))


def _in_maps(x, w_qkv, w_out):
    x = np.asarray(x, dtype=np.float32)
    w_qkv = np.asarray(w_qkv, dtype=np.float32)
    w_out = np.asarray(w_out, dtype=np.float32)
    maps = []
    for c in range(NCORES):
        b, qh = c // 2, c % 2
        r0 = qh * DQ

        def fold_m(a, m):  # [512, 256] -> [128, 512] (d-major, head-pair m)
            return np.concatenate(
                [a[128 * d:128 * (d + 1), 128 * m:128 * (m + 1)]
                 for d in range(4)], axis=1)

        wk = w_qkv[D + r0:D + r0 + DQ].T                # [512, 256]
        wq = w_qkv[r0:r0 + DQ].T
        wv = _fold(w_qkv[2 * D + r0:2 * D + r0 + DQ].T, 4)
        wo = _fold(w_out[:, r0:r0 + DQ].T, 2)           # [128, 1024]
        xT = x[b].T                                     # [512, 2048]
        maps.append({
            "wkqAT": np.concatenate(
                [fold_m(wk, 0), fold_m(wq, 0)], axis=1).astype(np.float16),
            "wkqBT": np.concatenate(
                [fold_m(wk, 1), fold_m(wq, 1)], axis=1).astype(np.float16),
            "wvT": wv.astype(np.float16),
            "woT": wo.astype(np.float16),
            **{f"xQ{r}": _fold(
                xT[:, r * 512:(r + 1) * 512], 4).astype(np.float16)
               for r in range(4)},
        })
    return maps


def _gather(results):
    out = np.empty((B, S, D), np.float32)
    for b in range(B):
        acc = np.zeros((512, 2048), np.float32)
        for c in (2 * b, 2 * b + 1):
            o = results[c]["outT"].astype(np.float32)   # [128, 8192]
            acc += o.reshape(128, 4, 2048).transpose(1, 0, 2).reshape(
                512, 2048)
        out[b] = acc.T
    return out


def run(x, w_qkv, w_out, trace=False):
    from concourse.bass_utils import run_bass_kernel_spmd

    nc = _get_nc()
    res = run_bass_kernel_spmd(
        nc, _in_maps(x, w_qkv, w_out), core_ids=list(range(NCORES)), trace=trace,
    )
    return _gather(res.results), res


def kernel(x, w_qkv, w_out):
    out, _ = run(x, w_qkv, w_out)
    return out


# revision 32
# speedup vs baseline: 1.1361x; 1.0025x over previous
"""Multi-head attention (B=4, S=2048, D=512, H=8) on 8 trn2 cores.

Sharding: core c handles batch b=c//2 and the head-quad qh=c%2 (heads
4*qh..4*qh+3). Each core computes q/k/v projections for its 4 heads over the
full sequence, flash-style attention (scores kept transposed [j, i] so all
matmul contractions land on the partition dim with zero on-device transposes),
and the partial output projection over its 256 o-dims. The host pre-packs
x/weight slices into sbuf-layout 2D dram tensors (free) and sums/transposes
the two partial outputs per batch.

Design (single fused pipeline, ~191us vs 204us for the v1 schedule; the
slot period is co-saturated: ACT exp ~1.10us busy/slot, PE scores+AV
~1.07us + ~0.2us/slot of drip-fed projection work):
 - 128 slots of [128,1024] exp are the ACT floor (~140us busy); the PE
   floor is slightly higher (scores+AV at the 128-outputs/cycle roofline
   plus ~0.1us/slot of stationary-swap drain the in-order self-loading
   matmul stream cannot hide, plus ~28us of projections). Everything is
   scheduled to keep both streams dense.
 - Scores are emitted one slot AHEAD of their exp (scores(s+1) before
   AV(s-4) in the PE stream) so the exp never sits on the scores-
   completion semaphore; at bufs=12 decouples the exp WAR from AV jitter.
 - Input DMAs: host packs weights and x into contiguous [128, N] dram
   tensors matching sbuf layout exactly (8KB/4KB descriptors, 9 triggers).
   The 16 DMA engines drain all queued transfers round-robin, so the
   non-critical half of the stream is token-gated behind the critical
   half's completion (GPSIMD copies into each gated dst) - first exp fires
   ~12us in instead of ~24us. A deeper gate chain measured faster on the
   best core but blows up tail-core variance; keep one level.
 - Unit order h0v0 h1v0 h0v1 h1v1 h2v0 h3v0 h2v1 h3v1 defers every m=1
   projection past slot 64, so the deadline-sorted deferred-work queue
   stays at <=1.5 quanta/slot (v-major order peaked at 2.4/slot in units
   0-2 and gapped the exp stream there). Boundary slots jc 0-1 carry two
   AV drains each and take no quanta; jc 2-3 carry none and take up to 3.
   PSUM: sp [128,1024]x2 + op [128,1024]x1 + sc [128,512]x2 = 8 banks.
 - Softmax normalization without DRAM round-trips: each v block carries 64
   ones-columns ([128,128] stationary = 64 ones | 64 v), so the AV matmul
   replicates the softmax denominator into op psum rows 0..63 for free.
   The epilogue reads op PSUM directly: reciprocal_approx_fast (base-0
   partitions, 18-bit exact; sums are ~[1,1e20], far from its denorm/inf
   edge cases) then one multiply; the next unit's AV start=True write
   WAR-waits on those two reads. Emitted at jc==1 of the next unit, right
   after the previous unit's last AV drains.
 - Tail: both epilogue halves back-to-back, outproj psum->sbuf casts
   alternate ACT/DVE (both idle there; Copy shares the exp act table so no
   reload), and each output scq ships as two 256KB m-half DMAs so the
   first half is in flight while the second half's casts run.
 - fp16 for the score path (x, w_qkv, q, k, w_out, o): 1 cycle/row on the
   PE like bf16 but 8x the mantissa (bf16 q/k fails the 2e-2 gate at
   ~2.2e-2; fp16 lands at 3.3e-3). exp output (attn weights) stays bf16
   for fp32 exponent range since softmax skips max-subtraction, psum stays
   fp32. Output partials are written fp16 (host sums in fp32), halving the
   output DMA.
"""
import sys

sys.path.insert(0, "/opt/trn_rl_repo")
import numpy as np

B, S, D, H, HD = 4, 2048, 512, 8, 64
HPC = 4          # heads per core
DQ = HPC * HD    # 256 projection dims per core
NCORES = 8
VW = 2 * HD      # v block width: 64 v-dims + 64 ones columns (128)
IH = S // 2      # i-half processed per attention unit (1024)
AV_LAG = 4       # attn@v trails scores by this many j-chunks

_cache = {}


def _build_nc():
    import concourse.bacc as bacc
    import concourse.mybir as mybir
    import concourse.tile as tile

    F32 = mybir.dt.float32
    F16, BF16 = mybir.dt.float16, mybir.dt.bfloat16
    EXP = mybir.ActivationFunctionType.Exp

    nc = bacc.Bacc("TRN2", target_bir_lowering=False, debug=False)

    # host-packed inputs, all contiguous 2D [128, N] in final sbuf layout:
    # wkqA/B: [k d-blocks (4x128) | q d-blocks (4x128)] for head-pair m=0/1
    # wvT: [v d-blocks (4x256)]
    # woT: [wo kc-blocks (2x512)]
    # xQ0..3: x column-quarters, d-major blocks of 512 cols each
    wkqAT = nc.dram_tensor("wkqAT", [128, 1024], F16, kind="ExternalInput")
    wkqBT = nc.dram_tensor("wkqBT", [128, 1024], F16, kind="ExternalInput")
    wvT = nc.dram_tensor("wvT", [128, 1024], F16, kind="ExternalInput")
    woT = nc.dram_tensor("woT", [128, 1024], F16, kind="ExternalInput")
    xQ = [nc.dram_tensor(f"xQ{r}", [128, 2048], F16, kind="ExternalInput")
          for r in range(4)]
    # output: block m (cols m*2048+s) holds out-dims m*128..m*128+127
    outT = nc.dram_tensor("outT", [128, 8192], F16, kind="ExternalOutput")

    with tile.TileContext(nc) as tc:
        with tc.tile_pool(name="sb", bufs=1) as sb, \
             tc.tile_pool(name="ps", bufs=1, space="PSUM") as pp:
            # ---- persistent sbuf tensors ----
            wkq = [sb.tile([128, 1024], F16, tag=f"wkq{m}", name=f"wkq{m}")
                   for m in range(2)]
            wvs = sb.tile([128, 1024], F16, tag="wvs", name="wvs")
            wos = sb.tile([128, 1024], F16, tag="wos", name="wos")
            xtf = sb.tile([128, 8192], F16, tag="xtf", name="xtf")

            def wk_v(d, m):      # k weights, d-block, head-pair m
                return wkq[m][:, d * 128:(d + 1) * 128]

            def wq_v(d, m):
                return wkq[m][:, 512 + d * 128:512 + (d + 1) * 128]

            def wv_v(d, hp):     # v weights, d-block, head-pair hp
                return wvs[:, d * 256 + hp * 128:d * 256 + (hp + 1) * 128]

            def wo_v(kc, m):
                return wos[:, kc * 512 + m * 128:kc * 512 + (m + 1) * 128]

            def xts(d, c0, c1):  # x cols c0:c1 of d-block (one 512-region)
                r = c0 // 512
                assert (c1 - 1) // 512 == r
                base = r * 2048 + d * 512 + (c0 - r * 512)
                return xtf[:, base:base + (c1 - c0)]

            # ---- input DMAs ----
            # The 16 DMA engines service all queued transfers round-robin by
            # descriptor, so every queued dma_start completes only near the
            # end of the whole in-flight stream. Free-run only what the
            # prologue needs (wkqA, xQ0, wv, xQ1 = 1.5MB); gate the rest
            # behind an xQ1-completion token (tiny GPSIMD copies into each
            # gated dst chain the gated DMA's WAR dependency to xQ1). A
            # deeper 3-level chain measured FASTER on the best core but blew
            # up tail-core variance (late levels miss defq deadlines), so
            # stay at one level.
            nc.sync.dma_start(out=wkq[0][:], in_=wkqAT[:, :])
            nc.sync.dma_start(out=xtf[:, 0:2048], in_=xQ[0][:, :])
            nc.sync.dma_start(out=wvs[:], in_=wvT[:, :])
            nc.sync.dma_start(out=xtf[:, 2048:4096], in_=xQ[1][:, :])
            tok1 = xtf[0:1, 2048:2049]    # written by xQ1
            for dst in (xtf[0:1, 4096:4097], xtf[0:1, 6144:6145],
                        wkq[1][0:1, 0:1], wos[0:1, 0:1]):
                nc.gpsimd.tensor_copy(out=dst, in_=tok1)
            nc.sync.dma_start(out=xtf[:, 4096:6144], in_=xQ[2][:, :])
            nc.sync.dma_start(out=xtf[:, 6144:8192], in_=xQ[3][:, :])
            nc.sync.dma_start(out=wkq[1][:], in_=wkqBT[:, :])
            nc.sync.dma_start(out=wos[:], in_=woT[:, :])

            qT = [sb.tile([128, S], F16, tag=f"qT{m}", name=f"qT{m}")
                  for m in range(2)]
            kT = [sb.tile([128, S], F16, tag=f"kT{m}", name=f"kT{m}")
                  for m in range(2)]
            # vv block for (jc, h): cols [0:64] = ones, [64:128] = v dims
            # (ones first so the softmax sums land at psum partitions 0:64,
            # where the base-0 custom-DVE fast reciprocal can read them)
            vv = sb.tile([128, 16 * HPC * VW], BF16, tag="vv", name="vv")
            # oTn[p]: heads (2p, 2p+1) stacked on partitions; outproj moving
            oTn = [sb.tile([128, S], F16, tag=f"oTn{p}", name=f"oTn{p}")
                   for p in range(2)]

            # ---- prologue scratch: ACT table preload + PE warm-up ----
            wub = sb.tile([128, 512], BF16, tag="wub", name="wub")
            nc.vector.memset(wub[:], 0.25)
            # tiny exp: forces the ACT exp table load off the critical path
            dummy_at = sb.tile([128, 16], BF16, tag="dummy_at", name="dummy_at")
            nc.scalar.activation(dummy_at[:], wub[:, 0:16], EXP)
            ones32 = sb.tile([128, 1], F32, tag="ones32", name="ones32")
            nc.vector.memset(ones32[:], 1.0)
            vv_ones = vv[:, :].rearrange("p (g w) -> p g w", w=VW)[:, :, 0:HD]
            nc.vector.tensor_copy(
                out=vv_ones, in_=ones32[:].to_broadcast((128, 16 * HPC, HD)))
            # keep the PE busy/ramping (HAM warm) while the critical input
            # DMAs stream in; 256-wide quanta so the tail of the warm-up
            # doesn't delay the first projection once data lands
            for _ in range(20):
                wups = pp.tile([128, 512], F32, tag="sc", bufs=2, name="wups")
                nc.tensor.matmul(wups[:, 0:256], wub[:, 0:128],
                                 wub[:, 0:256],
                                 start=True, stop=True, skip_group_check=True)

            # ---- work-group emitters ----
            def qk_cast(nm, m, sc, ps, cast_eng="dve"):
                tgt = (qT if nm == "q" else kT)[m]
                if cast_eng == "act":
                    # ACT is idle in the prologue; Copy shares the exp act
                    # table so there is no table reload
                    nc.scalar.activation(
                        tgt[:, sc * 512:(sc + 1) * 512], ps[:, 0:512],
                        mybir.ActivationFunctionType.Copy)
                else:
                    nc.vector.tensor_copy(
                        out=tgt[:, sc * 512:(sc + 1) * 512], in_=ps[:, 0:512])
                qk_done.add((nm, m, sc))

            def qk_group(nm, m, sc, cast_eng="dve"):
                """whole q/k projection group: 4 matmuls + cast (prologue)."""
                ps = pp.tile([128, 512], F32, tag="sc", bufs=2, name="ps")
                wf = wq_v if nm == "q" else wk_v
                for d in range(4):
                    nc.tensor.matmul(
                        ps[:, 0:512], wf(d, m),
                        xts(d, sc * 512, (sc + 1) * 512),
                        start=(d == 0), stop=(d == 3))
                qk_cast(nm, m, sc, ps, cast_eng)

            def qk_quanta(nm, m, sc, deadline):
                """same group cut into 4 one-matmul quanta for the defq,
                with staggered deadlines so pops stay ~1/slot."""
                state = {}
                def q(d):
                    def emit():
                        if d == 0:
                            state["ps"] = pp.tile([128, 512], F32, tag="sc",
                                                  bufs=2, name="psq")
                        ps = state["ps"]
                        wf = wq_v if nm == "q" else wk_v
                        nc.tensor.matmul(
                            ps[:, 0:512], wf(d, m),
                            xts(d, sc * 512, (sc + 1) * 512),
                            start=(d == 0), stop=(d == 3),
                            skip_group_check=True)
                        if d == 3:
                            qk_cast(nm, m, sc, ps)
                    return emit
                return [(deadline - (3 - d), q(d)) for d in range(4)]

            def v_emit(hp, jc):
                """v projection for head-pair hp, j-chunk jc (prologue)."""
                ps = pp.tile([128, 512], F32, tag="sc", bufs=2, name="psv")
                for d in range(4):
                    nc.tensor.matmul(
                        ps[:, 0:128], xts(d, jc * 128, (jc + 1) * 128),
                        wv_v(d, hp),
                        start=(d == 0), stop=(d == 3),
                        skip_group_check=True)
                base = jc * HPC * VW + hp * 2 * VW
                out_view = vv[:, base:base + 2 * VW].rearrange(
                    "p (h w) -> p h w", w=VW)[:, :, HD:VW]
                nc.vector.tensor_copy(
                    out=out_view,
                    in_=ps[:, 0:128].rearrange("p (h d) -> p h d", d=HD))
                vv_done.add((hp, jc))

            def v_quanta(hp, jc, deadline):
                """same, split into two 2-matmul quanta for smoothing."""
                state = {}
                def q(g):
                    def emit():
                        if g == 0:
                            state["ps"] = pp.tile([128, 512], F32, tag="sc",
                                                  bufs=2, name="psv")
                        ps = state["ps"]
                        for d in (2 * g, 2 * g + 1):
                            nc.tensor.matmul(
                                ps[:, 0:128], xts(d, jc * 128, (jc + 1) * 128),
                                wv_v(d, hp),
                                start=(d == 0), stop=(d == 3),
                                skip_group_check=True)
                        if g == 1:
                            base = jc * HPC * VW + hp * 2 * VW
                            out_view = vv[:, base:base + 2 * VW].rearrange(
                                "p (h w) -> p h w", w=VW)[:, :, HD:VW]
                            nc.vector.tensor_copy(
                                out=out_view,
                                in_=ps[:, 0:128].rearrange(
                                    "p (h d) -> p h d", d=HD))
                            vv_done.add((hp, jc))
                    return emit
                return [(deadline - 1, q(0)), (deadline, q(1))]

            # per-scq gather buffers for the output projection: the 4
            # m-groups of one scq cast into one [128, 2048] tile, then ONE
            # dma_start ships all 512KB (one Sync trigger instead of 4)
            ob_state = {}
            # emission-order guards: the tile tracker links readers only to
            # PRIOR writes, so a consumer emitted before its producer is a
            # silent race. Record what has been emitted; assert before use.
            vv_done = set()
            qk_done = set()

            def out_quanta(m, scq, deadline, pool_tag="sc", cast_eng="dve"):
                """output projection group: 2 matmul quanta + cast (+dma
                after the scq's 4th group)."""
                state = {}
                def q(kc):
                    def emit():
                        if kc == 0:
                            if pool_tag == "sp":
                                state["ps"] = pp.tile([128, IH], F32,
                                                      tag="sp", bufs=2,
                                                      name="psot")
                            else:
                                state["ps"] = pp.tile([128, 512], F32,
                                                      tag="sc", bufs=2,
                                                      name="pso")
                        ps = state["ps"]
                        nc.tensor.matmul(
                            ps[:, 0:512], wo_v(kc, m),
                            oTn[kc][:, scq * 512:(scq + 1) * 512],
                            start=(kc == 0), stop=(kc == 1),
                            skip_group_check=True)
                        if kc == 1:
                            if scq not in ob_state:
                                ob_state[scq] = [sb.tile(
                                    [128, 2048], F16, tag="ob", bufs=2,
                                    name=f"ob{scq}"), 0]
                            obt, _ = ob_state[scq]
                            dst = obt[:, m * 512:(m + 1) * 512]
                            if cast_eng == "act":
                                nc.scalar.activation(
                                    dst, ps[:, 0:512],
                                    mybir.ActivationFunctionType.Copy)
                            else:
                                nc.vector.tensor_copy(
                                    out=dst, in_=ps[:, 0:512])
                            ob_state[scq][1] += 1
                            # ship in m-halves so the first 256KB is in
                            # flight while the second half's casts run
                            if ob_state[scq][1] in (2, 4):
                                mh = ob_state[scq][1] // 2 - 1  # 0 or 1
                                nc.sync.dma_start(
                                    out=outT[:, :].rearrange(
                                        "p (m s) -> p m s", s=2048)[
                                        :, 2 * mh:2 * mh + 2,
                                        scq * 512:(scq + 1) * 512],
                                    in_=obt[:, mh * 1024:(mh + 1) * 1024]
                                    .rearrange("p (m s) -> p m s", s=512))
                    return emit
                return [(deadline, q(0)), (deadline, q(1))]

            # ---- prologue projections: everything unit (0,0) needs that
            # only depends on the first two x column-quarters ----
            with nc.named_scope("proj"):
                qk_group("k", 0, 0, cast_eng="act")
                qk_group("q", 0, 0)
                qk_group("q", 0, 1)

            # ---- deferred-work queue: (deadline_slot, emit) sorted ----
            # v(0, 0..2) used to be prologue emissions; as deadline-0..2
            # quanta they come off the exp(0) critical path (wv lands with
            # the free DMA set, so the pops never block the PE queue)
            defq = []
            for jc in range(3):
                defq += v_quanta(0, jc, jc)
            defq += qk_quanta("k", 0, 1, 4)      # scores(0,0) jc>=4
            defq += qk_quanta("k", 0, 2, 8)      # scores(0,0) jc>=8
            defq += qk_quanta("k", 0, 3, 12)
            # AV(u, jc) normally pops at slot jc+AV_LAG, but the unit
            # boundary drains the last 4 AVs early (2/slot at jc 0-1), so
            # clamp the tail deadlines below that
            for jc in range(3, 16):
                defq += v_quanta(0, jc, jc + AV_LAG if jc < 12 else jc - 1)
            # unit order defers all m=1 tensors to slot 64+, so the deferred
            # work spreads at <=1.5 pops/slot instead of 2.4 in units 0-2:
            #   units: h0v0 h1v0 | h0v1 h1v1 | h2v0 h3v0 | h2v1 h3v1
            defq += qk_quanta("q", 0, 2, 27)     # unit 2 scores @ slot 31
            defq += qk_quanta("q", 0, 3, 31)
            for jc in range(16):                 # vv hp1 for unit 4 @ 64+
                defq += v_quanta(1, jc, 34 + (27 * jc) // 16)
            defq += qk_quanta("q", 1, 0, 56)     # unit 4 scores @ slot 63
            defq += qk_quanta("q", 1, 1, 60)
            defq += qk_quanta("k", 1, 0, 61)
            defq += qk_quanta("k", 1, 1, 64)
            defq += qk_quanta("k", 1, 2, 68)
            defq += qk_quanta("k", 1, 3, 72)
            defq += qk_quanta("q", 1, 2, 88)     # unit 6 scores @ slot 95
            defq += qk_quanta("q", 1, 3, 92)
            defq.sort(key=lambda t: t[0])
            outproj_v0 = []   # gated on epilogue of unit 5 (h3,v0) ~slot 97
            for scq in range(2):      # scq-major so the combined per-scq
                for m in range(4):    # DMA fires right after its 4th group
                    outproj_v0 += out_quanta(m, scq, 120)
            outproj_v1 = []   # tail: needs the last unit's epilogue.
            # scq-major: the first half only reads columns the first
            # epilogue-half has normalized; casts alternate ACT/DVE (both
            # idle in the tail) so the cast chain halves
            gi = 0
            for scq in range(2, 4):
                for m in range(4):
                    outproj_v1 += out_quanta(
                        m, scq, 999, pool_tag=("sp" if gi % 2 else "sc"),
                        cast_eng=("act" if m % 2 == 0 else "dve"))
                    gi += 1

            # ---- attention: unit order keeps head-pair 0 first (defers the
            # m=1 projections), v0 before v1 within each pair so outproj
            # halves unlock as early as possible ----
            units = [(0, 0), (1, 0), (0, 1), (1, 1),
                     (2, 0), (3, 0), (2, 1), (3, 1)]
            nunits = len(units)

            def epilogue(uid, op, c0=0, c1=IH):
                """normalize straight out of op psum: rows 0..63 hold the
                softmax denominator replicated via the ones-columns of vv,
                rows 64..127 the unnormalized output. The next unit's AV
                start=True write WAR-waits on these two reads."""
                h, v = units[uid]
                p, off = h // 2, 64 * (h % 2)
                recip = sb.tile([64, IH], F32, tag="recip", bufs=2,
                                name="recip")
                nc.vector.reciprocal_approx_fast(
                    out=recip[:, c0:c1], in_=op[0:64, c0:c1])
                nc.vector.tensor_mul(
                    out=oTn[p][off:off + 64, v * IH + c0:v * IH + c1],
                    in0=op[64:128, c0:c1], in1=recip[:, c0:c1])

            with nc.named_scope("attn"):
                pend_av = []       # (uid, h, jc, at_tile, op_tile-or-None)
                op_t = [None]      # current unit's op accumulator
                op_prev = [None]
                sp_pend = {}       # s -> sp tile (scores emitted, exp not)

                def emit_scores(s):
                    """scores [j(128), i(1024)] for global slot s. Emitted
                    one slot AHEAD of its exp so the exp never waits on the
                    scores-completion semaphore (~150ns/slot otherwise)."""
                    uu, jj = s // 16, s % 16
                    hh, vv_ = units[uu]
                    mm, oo, ii0 = hh // 2, 64 * (hh % 2), vv_ * IH
                    assert ("k", mm, jj // 4) in qk_done, ("k", s, mm, jj)
                    sp = pp.tile([128, IH], F32, tag="sp", bufs=2,
                                 name="sp")
                    for scc in range(2):
                        assert ("q", mm, (ii0 + scc * 512) // 512) in qk_done, \
                            ("q", s, mm, ii0, scc)
                        nc.tensor.matmul(
                            sp[:, scc * 512:(scc + 1) * 512],
                            kT[mm][oo:oo + 64, jj * 128:(jj + 1) * 128],
                            qT[mm][oo:oo + 64,
                                   ii0 + scc * 512:ii0 + (scc + 1) * 512],
                            start=True, stop=True)
                    sp_pend[s] = sp

                def emit_av():
                    _, hh, jj, aa, oo = pend_av.pop(0)
                    if oo is None:
                        oo = op_t[0]
                    assert (hh // 2, jj) in vv_done, ("vv", hh, jj)
                    base = jj * HPC * VW + hh * VW
                    for scc in range(2):
                        nc.tensor.matmul(
                            oo[:, scc * 512:(scc + 1) * 512],
                            vv[:, base:base + VW],
                            aa[:, scc * 512:(scc + 1) * 512],
                            start=(jj == 0), stop=(jj == 15),
                            skip_group_check=True)

                emit_scores(0)
                for ui, (h, v) in enumerate(units):
                    for jc in range(16):
                        s = ui * 16 + jc
                        at_t = sb.tile([128, IH], BF16, tag="at", bufs=14,
                                       name="at")
                        nc.scalar.activation(at_t[:], sp_pend.pop(s), EXP)
                        if s + 1 < 16 * nunits:
                            emit_scores(s + 1)
                        # acquire op right before this unit's first AV (and
                        # after the previous unit's epilogue was emitted)
                        if jc == AV_LAG:
                            op_t[0] = pp.tile([128, IH], F32, tag="op",
                                              bufs=1, name="op")
                        pend_av.append(
                            (ui, h, jc, at_t,
                             op_t[0] if jc >= AV_LAG else None))
                        if len(pend_av) > AV_LAG:
                            emit_av()
                        # boundary: drain the previous unit's AVs 2/slot,
                        # then emit its epilogue right after its last AV so
                        # the op psum is free before this unit's first AV
                        # (popped at jc==AV_LAG) needs the banks
                        if (ui > 0 and jc < 2 and pend_av
                                and pend_av[0][0] == ui - 1):
                            emit_av()
                            if jc == 1:
                                while pend_av and pend_av[0][0] == ui - 1:
                                    emit_av()
                                epilogue(ui - 1, op_prev[0])
                        # last unit: pre-drain so the tail chain is short
                        if (ui == nunits - 1 and jc >= 12
                                and len(pend_av) > 1):
                            emit_av()
                        # deferred proj/outproj quanta in the PE slack,
                        # AFTER the AV pair: the slot's last matmul then
                        # precedes the next slot's scores-LS in issue order,
                        # giving every stationary load a full matmul of
                        # cover. Boundary slots jc 0-1 carry 2 AV drains
                        # each (PE ~1.6us, over budget) -> no pops there;
                        # jc 2-3 carry no AV at all -> up to 3 pops.
                        popped = 0
                        cap = 1
                        if ui > 0 and jc < 2:
                            cap = 0
                        elif ui > 0 and jc in (2, 3):
                            cap = 3
                        while defq and (popped < cap
                                        or defq[0][0] <= s + (2 if cap else 0)):
                            defq.pop(0)[1]()
                            popped += 1
                        if not defq and outproj_v0 and s >= 99:
                            want = min(cap - popped, len(outproj_v0))
                            for _ in range(max(0, want)):
                                outproj_v0.pop(0)[1]()
                    op_prev[0] = op_t[0]

                # drain the final unit's pipeline; both epilogue halves
                # first so the DVE normalization chain runs back-to-back
                # (nothing else needs the op psum), then the v0 leftovers
                # keep the PE hot until the v1 output projection unblocks
                while pend_av:
                    emit_av()
                last = nunits - 1
                epilogue(last, op_prev[0], 0, 512)
                epilogue(last, op_prev[0], 512, IH)
                for _, q in outproj_v0:
                    q()

            # ---- tail: remaining output projection ----
            with nc.named_scope("outproj"):
                for _, q in outproj_v1:
                    q()

    nc.compile()
    return nc


def _get_nc():
    if "nc" not in _cache:
        _cache["nc"] = _build_nc()
    return _cache["nc"]


def _fold(a, nblk):
    """[nblk*128, C] -> [128, nblk*C] with d-major column blocks."""
    r, c = a.shape
    assert r == nblk * 128
    return np.ascontiguousarray(
        a.reshape(nblk, 128, c).transpose(1, 0, 2).reshape(128, nblk * c))


def _in_maps(x, w_qkv, w_out):
    x = np.asarray(x, dtype=np.float32)
    w_qkv = np.asarray(w_qkv, dtype=np.float32)
    w_out = np.asarray(w_out, dtype=np.float32)
    maps = []
    for c in range(NCORES):
        b, qh = c // 2, c % 2
        r0 = qh * DQ

        def fold_m(a, m):  # [512, 256] -> [128, 512] (d-major, head-pair m)
            return np.concatenate(
                [a[128 * d:128 * (d + 1), 128 * m:128 * (m + 1)]
                 for d in range(4)], axis=1)

        wk = w_qkv[D + r0:D + r0 + DQ].T                # [512, 256]
        wq = w_qkv[r0:r0 + DQ].T
        wv = _fold(w_qkv[2 * D + r0:2 * D + r0 + DQ].T, 4)
        wo = _fold(w_out[:, r0:r0 + DQ].T, 2)           # [128, 1024]
        xT = x[b].T                                     # [512, 2048]
        maps.append({
            "wkqAT": np.concatenate(
                [fold_m(wk, 0), fold_m(wq, 0)], axis=1).astype(np.float16),
            "wkqBT": np.concatenate(
                [fold_m(wk, 1), fold_m(wq, 1)], axis=1).astype(np.float16),
            "wvT": wv.astype(np.float16),
            "woT": wo.astype(np.float16),
            **{f"xQ{r}": _fold(
                xT[:, r * 512:(r + 1) * 512], 4).astype(np.float16)
               for r in range(4)},
        })
    return maps


def _gather(results):
    out = np.empty((B, S, D), np.float32)
    for b in range(B):
        acc = np.zeros((512, 2048), np.float32)
        for c in (2 * b, 2 * b + 1):
            o = results[c]["outT"].astype(np.float32)   # [128, 8192]
            acc += o.reshape(128, 4, 2048).transpose(1, 0, 2).reshape(
                512, 2048)
        out[b] = acc.T
    return out


def run(x, w_qkv, w_out, trace=False):
    from concourse.bass_utils import run_bass_kernel_spmd

    nc = _get_nc()
    res = run_bass_kernel_spmd(
        nc, _in_maps(x, w_qkv, w_out), core_ids=list(range(NCORES)), trace=trace,
    )
    return _gather(res.results), res


def kernel(x, w_qkv, w_out):
    out, _ = run(x, w_qkv, w_out)
    return out
